# revision 9
# baseline (speedup 1.0000x reference)
"""Self-contained Trainium2 Bass kernel for nn_GNNEncoder (GCN message passing).

Strategy: partition graphs (and their node/edge slices) across 8 NeuronCores.
Each core owns a contiguous range of graphs; nodes are degree-sorted within
the core.  Per GCN layer: each core transforms its node slice (h = x @ W,
scaled by dis = 1/sqrt(deg)), AllGathers the bf16 node table, then runs the
full gather / segment-sum locally for its destination nodes using batched
dma_gather (256B rows) plus fixed selection-matrix matmuls on the PE array
(slot p -> psum row p//4).  Out-of-budget messages are aggregated through
"virtual node" partial sums (pass-2 staircase).  Mean-pooling per graph is a
matmul with a host-built one-hot membership matrix; the final MLP runs on
the pooled [ngraph, 64] tiles.  Output is assembled on the host.
"""
import sys

sys.path.insert(0, "/opt/trn_rl_repo")

import hashlib

import numpy as np

import concourse.bass as bass
import concourse.bacc as bacc
import concourse.tile as tile
from concourse import mybir
from concourse.bass_utils import run_bass_kernel_spmd
from concourse.tile_rust import add_dep_helper


NCORES = 8
FORCE_QUEUE = None  # set to an int to pin all SWDGE gathers to one queue (sim)
P = 128
D = 64
ELEM = 128          # bf16 elems per table row (256B)
WIN = 32768
GCAP = 384          # max graphs per core (3 tiles of 128)


def _wrap_idx(flat):
    """[num] -> [128, num/16] int16 wrapped: idx i at [i%16, i//16], tiled x8."""
    num = flat.size
    assert num % 16 == 0, num
    a = np.zeros((16, num // 16), dtype=np.int16)
    a[np.arange(num) % 16, np.arange(num) // 16] = flat.astype(np.int16)
    return np.tile(a, (8, 1))


def build_layout(edge_index, batch, G=2500):
    """Returns dict with global structure + per-core data arrays."""
    N = batch.shape[0]
    E = edge_index.shape[1]
    src_o, dst_o = np.asarray(edge_index[0]), np.asarray(edge_index[1])
    batch = np.asarray(batch)

    # ---- partition graphs across cores by balancing node counts ----
    gcnt = np.bincount(batch, minlength=G)          # nodes per graph
    gstart_node = np.concatenate([[0], np.cumsum(gcnt)])
    bounds = [0]
    for c in range(1, NCORES):
        target = round(N * c / NCORES)
        g = int(np.searchsorted(gstart_node, target))
        g = min(max(g, bounds[-1]), G)
        bounds.append(g)
    bounds.append(G)
    g_of_core = [(bounds[c], bounds[c + 1]) for c in range(NCORES)]
    n_c = [int(gstart_node[ge] - gstart_node[gb]) for gb, ge in g_of_core]
    ng_c = [ge - gb for gb, ge in g_of_core]
    assert max(ng_c) <= GCAP, ng_c

    nstrips = (max(n_c) + P - 1) // P
    CHUNK = (nstrips + 1) * P                        # +1 guaranteed zero strip
    R_TOT = NCORES * CHUNK
    win = min(WIN, R_TOT)
    HI_BASE = R_TOT - win

    # ---- in-degree (for sorting) ----
    indeg = np.bincount(dst_o, minlength=N)

    # ---- per-core node permutation: old node id -> (core, rank) ----
    node_core = np.empty(N, np.int32)
    for c, (gb, ge) in enumerate(g_of_core):
        node_core[gstart_node[gb]:gstart_node[ge]] = c
    new_gid = np.empty(N, np.int64)                  # old id -> new global id
    core_nodes_old = []                              # per core: old ids in rank order
    for c in range(NCORES):
        lo, hi = int(gstart_node[bounds[c]]), int(gstart_node[bounds[c + 1]])
        old_ids = np.arange(lo, hi)
        order = np.argsort(-indeg[old_ids], kind="stable")
        ranked = old_ids[order]
        core_nodes_old.append(ranked)
        new_gid[ranked] = c * CHUNK + np.arange(ranked.size)

    # ---- per-core edge lists bucketed by dst, split lo/hi by src new id ----
    src_n = new_gid[src_o]
    dst_c = node_core[dst_o]
    dst_rank = (new_gid[dst_o] % CHUNK).astype(np.int64)
    is_lo = src_n < win

    # per core: lists indexed by dst rank
    deg_lo = np.zeros((NCORES, nstrips * P), np.int32)
    deg_hi = np.zeros((NCORES, nstrips * P), np.int32)
    np.add.at(deg_lo, (dst_c, dst_rank), is_lo)
    np.add.at(deg_hi, (dst_c, dst_rank), ~is_lo)

    # sort edges by (core, dst_rank, lo/hi) for slot filling
    order = np.lexsort((~is_lo, dst_rank, dst_c))
    e_src = src_n[order]
    e_core = dst_c[order]
    e_rank = dst_rank[order]
    e_islo = is_lo[order]
    # per (core,dst): start offsets into sorted list
    tot_deg = deg_lo + deg_hi
    dst_off = np.zeros((NCORES, nstrips * P + 1), np.int64)
    for c in range(NCORES):
        dst_off[c, 1:] = np.cumsum(tot_deg[c])
        if c > 0:
            dst_off[c] += dst_off[c - 1, -1]

    # ---- choose per-strip budgets KL[s], KH[s] (uniform across cores) ----
    KL = np.zeros(nstrips, np.int32)
    KH = np.zeros(nstrips, np.int32)
    for s in range(nstrips):
        sl = slice(s * P, (s + 1) * P)
        for (deg, K) in ((deg_lo, KL), (deg_hi, KH)):
            d = deg[:, sl].ravel()                   # 8*128 counts
            best, bestc = 0, None
            for k in range(0, int(d.max()) + 4, 4):
                cost = NCORES * P * k + 3.0 * np.maximum(d - k, 0).sum()
                if bestc is None or cost < bestc:
                    best, bestc = k, cost
            K[s] = best
    NBL = KL // 4                                    # lo blocks per window
    NBH = KH // 4

    # ---- fill main slots + collect overflow (vnodes) ----
    lo_blocks_per_strip = NBL * 4                    # per strip (4 windows)
    hi_blocks_per_strip = NBH * 4
    CAP_LO = int(lo_blocks_per_strip.sum()) * P
    CAP_HI = int(hi_blocks_per_strip.sum()) * P
    CAP_VB = nstrips * 2 * P                         # 2 vnode blocks per strip

    lo_col0 = np.concatenate([[0], np.cumsum(lo_blocks_per_strip)])
    hi_col0 = np.concatenate([[0], np.cumsum(hi_blocks_per_strip)])
    CAP_LO = max(CAP_LO, P)
    CAP_HI = max(CAP_HI, P)

    idx_lo = np.zeros((NCORES, CAP_LO), np.int64)    # default 0 -> a pad row? see below
    idx_hi = np.zeros((NCORES, CAP_HI), np.int64)
    idx_vb = np.zeros((NCORES, CAP_VB), np.int64)    # 0 = T_vn zero row

    # zero rows: lo window: core0 chunk rows [n_c0, CHUNK) are zero; use CHUNK-1.
    ZLO = CHUNK - 1
    assert ZLO < win
    ZHI = R_TOT - 1 - HI_BASE                        # core7 last pad row, hi-window-relative
    idx_lo[:] = ZLO
    idx_hi[:] = ZHI

    # vnode assignment: per core, list of (strip, q, kind, msgs)
    vn_msgs = [[] for _ in range(NCORES)]            # per core: list of (count, [srcs], slotpos)
    for c in range(NCORES):
        for s in range(nstrips):
            kl, kh = int(KL[s]), int(KH[s])
            for r in range(P):
                dstr = s * P + r
                nlo, nhi = int(deg_lo[c, dstr]), int(deg_hi[c, dstr])
                if nlo + nhi == 0:
                    continue
                base = int(dst_off[c, dstr])
                srcs = e_src[base : base + nlo + nhi]
                w, q = divmod(r, 32)
                # lo slots
                take = min(nlo, kl)
                for k in range(take):
                    b, t = divmod(k, 4)
                    col = lo_col0[s] + w * (kl // 4) + b
                    idx_lo[c, col * P + q * 4 + t] = srcs[k]
                if nlo > kl:
                    vn_msgs[c].append((nlo - kl, srcs[kl:nlo], (s, r, 0)))
                # hi slots
                take = min(nhi, kh)
                for k in range(take):
                    b, t = divmod(k, 4)
                    col = hi_col0[s] + w * (kh // 4) + b
                    idx_hi[c, col * P + q * 4 + t] = srcs[nlo + k] - HI_BASE
                if nhi > kh:
                    vn_msgs[c].append((nhi - kh, srcs[nlo + kh:], (s, r, 1)))

    # ---- pass-2: vnodes sorted by count desc, staircase strips ----
    nvn = max((len(v) for v in vn_msgs), default=0)
    NVSTRIP = max(1, (nvn + P - 1) // P)
    # per vnode-strip: number of staircase blocks (uniform across cores)
    vb_counts = np.zeros((NCORES, NVSTRIP * P), np.int32)
    for c in range(NCORES):
        vn_msgs[c].sort(key=lambda x: -x[0])
        for i, (cnt, _, _) in enumerate(vn_msgs[c]):
            vb_counts[c, i] = cnt
    NSB = np.zeros(NVSTRIP, np.int32)                # staircase blocks per vstrip
    for v in range(NVSTRIP):
        NSB[v] = int(vb_counts[:, v * P : (v + 1) * P].max())
    # pass-2 gathers: lo-kind vnodes and hi-kind in the SAME staircase
    # (mixed sources!) -> need separate lo/hi passes. Instead: two separate
    # staircases would double machinery; simpler: one staircase but each
    # slot's source window differs per vnode kind -> impossible per call.
    # Resolution: sort vnodes by (kind, -count): lo-vnodes first. Then
    # per strip, per block: slots [0, n_lo_valid) from lo window and
    # [n_lo..] from hi window -> two calls with complementary zero-pads.
    for c in range(NCORES):
        vn_msgs[c].sort(key=lambda x: (x[2][2], -x[0]))
    vb_counts[:] = 0
    vkind = np.zeros((NCORES, NVSTRIP * P), np.int32)
    for c in range(NCORES):
        for i, (cnt, _, _) in enumerate(vn_msgs[c]):
            vb_counts[c, i] = cnt
            vkind[c, i] = vn_msgs[c][i][2][2]
    for v in range(NVSTRIP):
        NSB[v] = int(vb_counts[:, v * P : (v + 1) * P].max())
    CAP_P2 = int(NSB.sum()) * P
    idx_p2lo = np.full((NCORES, max(CAP_P2, 16)), ZLO, np.int64)
    idx_p2hi = np.full((NCORES, max(CAP_P2, 16)), ZHI, np.int64)
    p2_col0 = np.concatenate([[0], np.cumsum(NSB)])
    for c in range(NCORES):
        for i, (cnt, srcs, (s, r, kind)) in enumerate(vn_msgs[c]):
            v, p = divmod(i, P)
            for k in range(cnt):
                col = p2_col0[v] + k
                if kind == 0:
                    idx_p2lo[c, col * P + p] = srcs[k]
                else:
                    idx_p2hi[c, col * P + p] = srcs[k] - HI_BASE
            # main v-block slot for this vnode: T_vn row = 1 + i
            # strip s vnode blocks: cols [2s, 2s+2), slot p2 = 2*q + kind
            # where within-block: block = r//64, pos = (r%64)*2 + kind
            blk, rr = divmod(r, 64)
            idx_vb[c, (s * 2 + blk) * P + rr * 2 + kind] = 1 + i
    VCAP = NVSTRIP * P
    # pass-2 lo/hi column split: lo vnodes occupy leading rows (kind-major sort)
    last_lo_v, first_hi_v = -1, NVSTRIP
    for c in range(NCORES):
        for i, (cnt, _, (s_, r_, kind)) in enumerate(vn_msgs[c]):
            v = i // P
            if kind == 0:
                last_lo_v = max(last_lo_v, v)
            else:
                first_hi_v = min(first_hi_v, v)
    P2LO_NCOL = int(p2_col0[last_lo_v + 1]) if last_lo_v >= 0 else 0
    P2HI_COL0 = int(p2_col0[first_hi_v]) if first_hi_v < NVSTRIP else int(p2_col0[-1])

    # ---- degree / pooling data ----
    deg_arr = np.ones((NCORES, P, nstrips), np.float32)
    cnt_arr = np.ones((NCORES, P, 3), np.float32)
    spool = np.zeros((NCORES, P, nstrips * GCAP), np.float32)
    for c in range(NCORES):
        old = core_nodes_old[c]
        dg = (indeg[old] + 1).astype(np.float32)     # +1 self loop
        r = np.arange(old.size)
        deg_arr[c, r % P, r // P] = dg
        gb, ge = g_of_core[c]
        gl = (batch[old] - gb).astype(np.int64)      # local graph id per rank
        spool[c, r % P, (r // P) * GCAP + gl] = 1.0
        gcl = gcnt[gb:ge].astype(np.float32)
        gcl = np.maximum(gcl, 1.0)
        gi = np.arange(ge - gb)
        cnt_arr[c, gi % P, gi // P] = gcl

    return dict(
        N=N, G=G, NSTRIPS=nstrips, CHUNK=CHUNK, R_TOT=R_TOT, HI_BASE=HI_BASE, WIN=win,
        NBL=NBL, NBH=NBH, NVSTRIP=NVSTRIP, NSB=NSB, VCAP=VCAP,
        CAP_LO=CAP_LO, CAP_HI=CAP_HI, CAP_VB=CAP_VB, CAP_P2=max(CAP_P2, 16),
        P2LO_NCOL=P2LO_NCOL, P2HI_COL0=P2HI_COL0,
        lo_col0=lo_col0, hi_col0=hi_col0, p2_col0=p2_col0,
        g_of_core=g_of_core, ng_c=ng_c, n_c=n_c,
        core_nodes_old=core_nodes_old,
        idx_lo=idx_lo, idx_hi=idx_hi, idx_vb=idx_vb,
        idx_p2lo=idx_p2lo, idx_p2hi=idx_p2hi,
        deg=deg_arr, cnt=cnt_arr, spool=spool,
        wrap=_wrap_idx,
    )


def core_inputs(lay, c, x, W1, W2, W3, Wp1, Wp2, b1, b2, b3, bp1, bp2):
    """Build the in_map for core c (numpy arrays, host dtypes)."""
    import ml_dtypes
    bf = ml_dtypes.bfloat16
    CHUNK, nstrips = lay["CHUNK"], lay["NSTRIPS"]
    old = lay["core_nodes_old"][c]
    xs = np.zeros((CHUNK, 128), np.float32)
    xs[: old.size] = x[old]
    w = lay["wrap"]
    S4 = np.zeros((P, 32), bf)
    for p in range(P):
        S4[p, p // 4] = 1.0
    S2 = np.zeros((P, 64), bf)
    for p in range(P):
        S2[p, p // 2] = 1.0
    I128b = np.eye(P, dtype=bf)
    I128f = np.eye(P, dtype=np.float32)
    return {
        "x": xs,
        "deg": lay["deg"][c],
        "cnt": lay["cnt"][c],
        "spool": lay["spool"][c].astype(bf),
        "idx_lo": w(lay["idx_lo"][c]),
        "idx_hi": w(lay["idx_hi"][c]),
        "idx_vb": w(lay["idx_vb"][c]),
        "idx_p2lo": w(lay["idx_p2lo"][c]),
        "idx_p2hi": w(lay["idx_p2hi"][c]),
        "s4": S4, "s2": S2, "i128b": I128b, "i128f": I128f,
        "W1": W1.astype(bf), "W2": W2.astype(bf), "W3": W3.astype(bf),
        "Wp1": Wp1.astype(bf), "Wp2": Wp2.astype(bf),
        "b1": b1.reshape(1, -1).astype(np.float32),
        "b2": b2.reshape(1, -1).astype(np.float32),
        "b3": b3.reshape(1, -1).astype(np.float32),
        "bp1": bp1.reshape(1, -1).astype(np.float32),
        "bp2": bp2.reshape(1, -1).astype(np.float32),
    }




bf16 = mybir.dt.bfloat16
f32 = mybir.dt.float32
i16 = mybir.dt.int16

AF = mybir.ActivationFunctionType
ALU = mybir.AluOpType

MAIN_CHUNK = 4      # strips per main gather chunk
P2_CHUNK = 32       # max pass-2 columns per gather chunk


def build_program(lay, ncores=8, has_bias=False, reps=1, ablate=()):
    NS = lay["NSTRIPS"]
    CHUNK = lay["CHUNK"]
    R_TOT = lay["R_TOT"]
    HI_BASE = lay["HI_BASE"]
    NBL, NBH = lay["NBL"], lay["NBH"]
    NVS = lay["NVSTRIP"]
    NSB = lay["NSB"]
    CAP_LO, CAP_HI, CAP_VB, CAP_P2 = (
        lay["CAP_LO"], lay["CAP_HI"], lay["CAP_VB"], lay["CAP_P2"])
    WINE = lay["WIN"]
    LO_NCOL = lay["P2LO_NCOL"]
    HI_COL0 = lay["P2HI_COL0"]
    lo_col0, hi_col0, p2_col0 = lay["lo_col0"], lay["hi_col0"], lay["p2_col0"]
    NCOL_P2 = int(NSB.sum())
    VROWS = NVS * P
    p2_chunk = max(P2_CHUNK, int(NSB.max()) if NVS else 0)
    max_lo = max(int(lo_col0[min(s + MAIN_CHUNK, NS)] - lo_col0[s])
                 for s in range(0, NS, MAIN_CHUNK))
    max_hi = max(int(hi_col0[min(s + MAIN_CHUNK, NS)] - hi_col0[s])
                 for s in range(0, NS, MAIN_CHUNK))

    nc = bacc.Bacc("TRN2", target_bir_lowering=False, num_devices=ncores,
                   num_swdge_queues=4)

    # ---------------- I/O ----------------
    x_in = nc.dram_tensor("x", [CHUNK, 128], f32, kind="ExternalInput")
    deg_in = nc.dram_tensor("deg", [P, NS], f32, kind="ExternalInput")
    cnt_in = nc.dram_tensor("cnt", [P, 3], f32, kind="ExternalInput")
    spool_in = nc.dram_tensor("spool", [P, NS * GCAP], bf16, kind="ExternalInput")
    idx_lo_in = nc.dram_tensor("idx_lo", [P, CAP_LO // 16], i16, kind="ExternalInput")
    idx_hi_in = nc.dram_tensor("idx_hi", [P, CAP_HI // 16], i16, kind="ExternalInput")
    idx_vb_in = nc.dram_tensor("idx_vb", [P, CAP_VB // 16], i16, kind="ExternalInput")
    idx_p2lo_in = nc.dram_tensor("idx_p2lo", [P, CAP_P2 // 16], i16, kind="ExternalInput")
    idx_p2hi_in = nc.dram_tensor("idx_p2hi", [P, CAP_P2 // 16], i16, kind="ExternalInput")
    s4_in = nc.dram_tensor("s4", [P, 32], bf16, kind="ExternalInput")
    s2_in = nc.dram_tensor("s2", [P, 64], bf16, kind="ExternalInput")
    i128b_in = nc.dram_tensor("i128b", [P, P], bf16, kind="ExternalInput")
    i128f_in = nc.dram_tensor("i128f", [P, P], f32, kind="ExternalInput")
    W_in = {
        "W1": nc.dram_tensor("W1", [128, 64], bf16, kind="ExternalInput"),
        "W2": nc.dram_tensor("W2", [64, 64], bf16, kind="ExternalInput"),
        "W3": nc.dram_tensor("W3", [64, 64], bf16, kind="ExternalInput"),
        "Wp1": nc.dram_tensor("Wp1", [64, 64], bf16, kind="ExternalInput"),
        "Wp2": nc.dram_tensor("Wp2", [64, 32], bf16, kind="ExternalInput"),
    }
    b_in = {
        "b1": nc.dram_tensor("b1", [1, 64], f32, kind="ExternalInput"),
        "b2": nc.dram_tensor("b2", [1, 64], f32, kind="ExternalInput"),
        "b3": nc.dram_tensor("b3", [1, 64], f32, kind="ExternalInput"),
        "bp1": nc.dram_tensor("bp1", [1, 64], f32, kind="ExternalInput"),
        "bp2": nc.dram_tensor("bp2", [1, 32], f32, kind="ExternalInput"),
    }
    z_out = nc.dram_tensor("z", [GCAP, 32], f32, kind="ExternalOutput")

    # NOTE: only the first D (=64) elems of each 256B table row carry data —
    # every aggregation matmul reads rhs cols [0:D) — so the collective moves
    # compact 128B rows (T_slice/T_ag_c) and a local spread DMA writes them
    # into the 256B-pitch gather table (upper halves are never-read garbage).
    T_slice = nc.dram_tensor("T_slice", [CHUNK, D], bf16)
    T_ag_c = nc.dram_tensor("T_ag_c", [R_TOT, D], bf16, addr_space="Shared")
    # double-buffered 256B-pitch table (gathers of layer L read parity L%2
    # while the next layer's spread writes the other parity)
    T_ag2 = [nc.dram_tensor(f"T_ag{par}", [R_TOT, ELEM], bf16) for par in range(2)]
    T_vn = nc.dram_tensor("T_vn", [1 + VROWS, ELEM], bf16)

    with tile.TileContext(nc) as tc:
        with (
            tc.tile_pool(name="const", bufs=1) as cp,
            tc.tile_pool(name="big", bufs=1) as bigp,
            tc.tile_pool(name="gat", bufs=3) as gp,
            tc.tile_pool(name="work", bufs=2) as wp,
            tc.tile_pool(name="ps", bufs=2, space="PSUM") as ps,
            tc.tile_pool(name="psb", bufs=1, space="PSUM") as psb,
            tc.tile_pool(name="pspool", bufs=1, space="PSUM") as psp,
        ):
            # ---------- load constants ----------
            def load(t_dram, shape, dtype, name):
                t = cp.tile(shape, dtype, tag=name)
                nc.sync.dma_start(out=t[:], in_=t_dram[:, :])
                return t

            idx_lo = load(idx_lo_in, [P, CAP_LO // 16], i16, "idxlo")
            idx_hi = load(idx_hi_in, [P, CAP_HI // 16], i16, "idxhi")
            idx_vb = load(idx_vb_in, [P, CAP_VB // 16], i16, "idxvb")
            idx_p2lo = load(idx_p2lo_in, [P, CAP_P2 // 16], i16, "idxp2lo")
            idx_p2hi = load(idx_p2hi_in, [P, CAP_P2 // 16], i16, "idxp2hi")
            s4 = load(s4_in, [P, 32], bf16, "s4")
            s2 = load(s2_in, [P, 64], bf16, "s2")
            i128b = load(i128b_in, [P, P], bf16, "i128b")
            i128f = load(i128f_in, [P, P], f32, "i128f")
            Wt = {k: load(v, [v.shape[0], v.shape[1]], bf16, k) for k, v in W_in.items()}
            bt = {k: load(v, [1, v.shape[1]], f32, k) for k, v in b_in.items()}
            deg = load(deg_in, [P, NS], f32, "deg")
            cnt = load(cnt_in, [P, 3], f32, "cnt")

            # dis = 1/sqrt(deg); invc = 1/cnt
            dtmp = cp.tile([P, NS], f32, tag="dtmp")
            nc.scalar.activation(out=dtmp[:], in_=deg[:], func=AF.Sqrt)
            dis = cp.tile([P, NS], f32, tag="dis")
            nc.vector.reciprocal(out=dis[:], in_=dtmp[:])
            invc = cp.tile([P, 3], f32, tag="invc")
            nc.vector.reciprocal(out=invc[:], in_=cnt[:])

            # big persistent buffers
            hsum = bigp.tile([P, NS * D], bf16, tag="hsum")
            nc.any.memset(hsum[:], 0.0)
            tstage = bigp.tile([P, NS * D], bf16, tag="tstage")
            nc.any.memset(tstage[:], 0.0)
            vzero = cp.tile([1, ELEM], bf16, tag="vzero")
            nc.any.memset(vzero[:], 0.0)
            nc.sync.dma_start(out=T_vn[0:1, :], in_=vzero[:])
            vtmp = bigp.tile([P, NVS * D], bf16, tag="vtmp")
            nc.any.memset(vtmp[:], 0.0)
            pre = bigp.tile([P, NS * D], f32, tag="pre")
            buf_vb = bigp.tile([P, NS * 2 * ELEM], bf16, tag="bvb")

            zpad = cp.tile([P, D], bf16, tag="zpad")
            nc.any.memset(zpad[:], 0.0)

            # collectives go through the dedicated CC pipeline (not SWDGE),
            # so all 4 SWDGE queues are available for immediate-mode gathers.
            # vb gathers wait on the whole pass-2 chain -> pin them to their
            # own queue so they never head-of-line-block the main gathers.
            qload = [0, 0, 0]
            VB_QUEUE = 3

            def next_q(n=1):
                if FORCE_QUEUE is not None:
                    return FORCE_QUEUE
                q = qload.index(min(qload))
                qload[q] += n
                return q

            def transform_strip(s, src_tile, src_slice, w_tile, fp32_in):
                """src rows [128 x k] -> tstage[:, s*ELEM : s*ELEM+64] = dis*(x@W)."""
                k = 128 if fp32_in else 64
                tp = ps.tile([k, 128], f32 if fp32_in else bf16, space="PSUM", tag="aux")
                nc.tensor.transpose(
                    out=tp[:, :], in_=src_tile[:, src_slice],
                    identity=(i128f if fp32_in else i128b)[:],
                )
                xT = wp.tile([k, 128], bf16, tag="xT")
                nc.vector.tensor_copy(out=xT[:], in_=tp[:, :])
                hn = psb.tile([P, D], f32, space="PSUM", tag="hn")
                nc.tensor.matmul(out=hn[:, :], lhsT=xT[:], rhs=w_tile[:],
                                 start=True, stop=True)
                nc.scalar.activation(
                    out=tstage[:, s * D : (s + 1) * D], in_=hn[:, :],
                    func=AF.Copy, scale=dis[:, s : s + 1],
                )

            for layer_it in range(3 * reps):
                layer = layer_it % 3 + 1
                T_ag = T_ag2[layer_it % 2]
                # ---------- phase A: build table (layer 1 only) ----------
                if layer_it == 0:
                    for s in range(NS):
                        xt = wp.tile([P, 128], f32, tag="xt")
                        nc.sync.dma_start(
                            out=xt[:], in_=x_in[s * P : (s + 1) * P, :])
                        transform_strip(s, xt, slice(0, 128), Wt["W1"], True)

                # ---------- phase B: export compact slice + AllGather ----------
                nc.sync.dma_start(
                    out=T_slice[0 : NS * P, :].rearrange("(s p) c -> p s c", p=P),
                    in_=tstage[:].rearrange("p (s c) -> p s c", c=D),
                )
                if NS * P < CHUNK and layer_it == 0:
                    # zero the pad strip rows once (values persist)
                    nc.sync.dma_start(
                        out=T_slice[NS * P : CHUNK, :]
                        .rearrange("(q p) c -> p q c", p=P),
                        in_=zpad[:].rearrange("p (q c) -> p q c", c=D)
                        .to_broadcast([P, (CHUNK - NS * P) // P, D]),
                    )
                if "nocc" not in ablate:
                    nc.gpsimd.collective_compute(
                        "AllGather", ALU.bypass,
                        ins=[T_slice[:, :]], outs=[T_ag_c[:, :]],
                        replica_groups=[list(range(ncores))],
                    )
                # spread compact rows into this layer's 256B-pitch table
                nc.sync.dma_start(out=T_ag[:, 0:D], in_=T_ag_c[:, :])

                # ---------- phase C: pass-2 vnode partial sums ----------
                if NCOL_P2 > 0 and "novn" not in ablate:
                    # chunk pass-2 columns by vstrips
                    v = 0
                    while v < NVS:
                        v0 = v
                        cols0 = int(p2_col0[v0])
                        while v < NVS and (v == v0 or int(p2_col0[v + 1]) - cols0 <= p2_chunk):
                            v += 1
                        cols1 = int(p2_col0[v])
                        ncol = cols1 - cols0
                        if ncol == 0:
                            v += 1
                            continue
                        # lo part of these columns
                        lo_c0, lo_c1 = cols0, min(cols1, LO_NCOL)
                        hi_c0, hi_c1 = max(cols0, HI_COL0), cols1
                        buf_l = buf_h = None
                        if lo_c1 > lo_c0:
                            n = 1 if "tinygather" in ablate else (lo_c1 - lo_c0)
                            buf_l = gp.tile([P, p2_chunk * ELEM], bf16, tag="p2l")
                            nc.gpsimd.dma_gather(
                                out_ap=buf_l[:, : n * ELEM].rearrange(
                                    "p (n d) -> p n d", d=ELEM),
                                in_ap=T_ag[0:WINE, :],
                                idxs_ap=idx_p2lo[:, lo_c0 * 8 : (lo_c0 + n) * 8],
                                num_idxs=n * P, num_idxs_reg=n * P,
                                elem_size=ELEM, queue_num=next_q(n * P), single_packet=False,
                            )
                        if hi_c1 > hi_c0:
                            n = 1 if "tinygather" in ablate else (hi_c1 - hi_c0)
                            buf_h = gp.tile([P, p2_chunk * ELEM], bf16, tag="p2h")
                            nc.gpsimd.dma_gather(
                                out_ap=buf_h[:, : n * ELEM].rearrange(
                                    "p (n d) -> p n d", d=ELEM),
                                in_ap=T_ag[HI_BASE : HI_BASE + WINE, :],
                                idxs_ap=idx_p2hi[:, hi_c0 * 8 : (hi_c0 + n) * 8],
                                num_idxs=n * P, num_idxs_reg=n * P,
                                elem_size=ELEM, queue_num=next_q(n * P), single_packet=False,
                            )
                        for vv in range(v0, v):
                            nblk = int(NSB[vv])
                            if nblk == 0:
                                continue
                            vps = ps.tile([P, D], f32, space="PSUM", tag="acc")
                            first = True
                            mms = []
                            for k in range(nblk):
                                col = int(p2_col0[vv]) + k
                                if col < LO_NCOL:
                                    mms.append((buf_l, col - lo_c0))
                                if col >= HI_COL0:
                                    mms.append((buf_h, col - hi_c0))
                            for mi, (buf, rel) in enumerate(mms):
                                nc.tensor.matmul(
                                    out=vps[:, :], lhsT=i128b[:],
                                    rhs=buf[:, rel * ELEM : rel * ELEM + D],
                                    start=(mi == 0), stop=(mi == len(mms) - 1),
                                    skip_group_check=True,
                                )
                            nc.scalar.activation(
                                out=vtmp[:, vv * D : (vv + 1) * D], in_=vps[:, :],
                                func=AF.Copy,
                            )
                    nc.sync.dma_start(
                        out=T_vn[1 : 1 + VROWS, 0:D].rearrange(
                            "(v p) c -> p v c", p=P),
                        in_=vtmp[:].rearrange("p (v c) -> p v c", c=D),
                    )

                # ---------- phase D: sweep 1 — lo/hi aggregation into pre ----------
                s = 0
                while s < NS:
                    s0, s1 = s, min(s + MAIN_CHUNK, NS)
                    s = s1
                    lc0, lc1 = int(lo_col0[s0]), int(lo_col0[s1])
                    hc0, hc1 = int(hi_col0[s0]), int(hi_col0[s1])
                    buf_lo = buf_hi = None
                    if lc1 > lc0:
                        n = 1 if "tinygather" in ablate else (lc1 - lc0)
                        buf_lo = gp.tile([P, max_lo * ELEM], bf16, tag="blo")
                        nc.gpsimd.dma_gather(
                            out_ap=buf_lo[:, : n * ELEM].rearrange(
                                "p (n d) -> p n d", d=ELEM),
                            in_ap=T_ag[0:WINE, :],
                            idxs_ap=idx_lo[:, lc0 * 8 : (lc0 + n) * 8],
                            num_idxs=n * P, num_idxs_reg=n * P,
                            elem_size=ELEM, queue_num=next_q(n * P), single_packet=False,
                        )
                    if hc1 > hc0:
                        n = 1 if "tinygather" in ablate else (hc1 - hc0)
                        buf_hi = gp.tile([P, max_hi * ELEM], bf16, tag="bhi")
                        nc.gpsimd.dma_gather(
                            out_ap=buf_hi[:, : n * ELEM].rearrange(
                                "p (n d) -> p n d", d=ELEM),
                            in_ap=T_ag[HI_BASE : HI_BASE + WINE, :],
                            idxs_ap=idx_hi[:, hc0 * 8 : (hc0 + n) * 8],
                            num_idxs=n * P, num_idxs_reg=n * P,
                            elem_size=ELEM, queue_num=next_q(n * P), single_packet=False,
                        )

                    for ss in range(s0, s1):
                        nbl, nbh = int(NBL[ss]), int(NBH[ss])
                        acc = ps.tile([P, D], f32, space="PSUM", tag="acc")
                        for w in range(4):
                            total = nbl + nbh
                            done = 0
                            for b in range(nbl):
                                col = int(lo_col0[ss]) - lc0 + w * nbl + b
                                nc.tensor.matmul(
                                    out=acc[32 * w : 32 * w + 32, :], lhsT=s4[:],
                                    rhs=buf_lo[:, col * ELEM : col * ELEM + D],
                                    start=(done == 0), stop=(done == total - 1),
                                    tile_position=(0, 32 * w),
                                    skip_group_check=True,
                                )
                                done += 1
                            for b in range(nbh):
                                col = int(hi_col0[ss]) - hc0 + w * nbh + b
                                nc.tensor.matmul(
                                    out=acc[32 * w : 32 * w + 32, :], lhsT=s4[:],
                                    rhs=buf_hi[:, col * ELEM : col * ELEM + D],
                                    start=(done == 0), stop=(done == total - 1),
                                    tile_position=(0, 32 * w),
                                    skip_group_check=True,
                                )
                                done += 1
                        # self-loop add: acc += tstage strip
                        nc.vector.tensor_tensor(
                            out=acc[:, :], in0=acc[:, :],
                            in1=tstage[:, ss * D : (ss + 1) * D],
                            op=ALU.add,
                        )
                        # pre[ss] = dis * acc (scaled lo/hi+self partial, f32)
                        nc.scalar.activation(
                            out=pre[:, ss * D : (ss + 1) * D], in_=acc[:, :],
                            func=AF.Copy, scale=dis[:, ss : ss + 1],
                        )

                # ---------- phase E: one vb gather + sweep 2 ----------
                if "novn" not in ablate:
                    n = 1 if "tinygather" in ablate else NS * 2
                    nc.gpsimd.dma_gather(
                        out_ap=buf_vb[:, : n * ELEM].rearrange(
                            "p (n d) -> p n d", d=ELEM),
                        in_ap=T_vn[:, :],
                        idxs_ap=idx_vb[:, 0 : n * 8],
                        num_idxs=n * P, num_idxs_reg=n * P,
                        elem_size=ELEM, queue_num=next_q(n * P), single_packet=False,
                    )
                for ss in range(NS):
                    xl = wp.tile([P, D], bf16, tag="xl")
                    if "novn" in ablate:
                        nc.scalar.activation(
                            out=xl[:], in_=pre[:, ss * D : (ss + 1) * D],
                            func=AF.Relu)
                    else:
                        acc2 = ps.tile([P, D], f32, space="PSUM", tag="acc")
                        for blk in range(2):
                            col = ss * 2 + blk
                            nc.tensor.matmul(
                                out=acc2[64 * blk : 64 * blk + 64, :], lhsT=s2[:],
                                rhs=buf_vb[:, col * ELEM : col * ELEM + D],
                                start=True, stop=True,
                                tile_position=(0, 64 * blk),
                                skip_group_check=True,
                            )
                        # ub = dis * acc2 + pre[ss]
                        ub = ps.tile([P, D], f32, space="PSUM", tag="aux")
                        nc.vector.scalar_tensor_tensor(
                            out=ub[:, :], in0=acc2[:, :],
                            scalar=dis[:, ss : ss + 1],
                            in1=pre[:, ss * D : (ss + 1) * D],
                            op0=ALU.mult, op1=ALU.add,
                        )
                        if has_bias:
                            nc.vector.tensor_tensor(
                                out=ub[:, :], in0=ub[:, :],
                                in1=bt[f"b{layer}"][:].to_broadcast([P, D]),
                                op=ALU.add)
                        nc.scalar.activation(out=xl[:], in_=ub[:, :], func=AF.Relu)
                    # hsum += x_l
                    nc.vector.tensor_tensor(
                        out=hsum[:, ss * D : (ss + 1) * D],
                        in0=hsum[:, ss * D : (ss + 1) * D],
                        in1=xl[:], op=ALU.add,
                    )
                    # next-layer table entry
                    if layer < 3 or reps > 1:
                        transform_strip(
                            ss, xl, slice(0, D),
                            Wt["W2" if layer == 3 else f"W{layer + 1}"], False)

            # ---------- pooling ----------
            pooled = []
            for t in range(3):
                pt = psp.tile([P, D], f32, space="PSUM", tag=f"pool{t}")
                pooled.append(pt)
            for ss in range(NS):
                spt = wp.tile([P, GCAP], bf16, tag="spt")
                nc.sync.dma_start(out=spt[:], in_=spool_in[:, ss * GCAP : (ss + 1) * GCAP])
                for t in range(3):
                    nc.tensor.matmul(
                        out=pooled[t][:, :], lhsT=spt[:, t * P : (t + 1) * P],
                        rhs=hsum[:, ss * D : (ss + 1) * D],
                        start=(ss == 0), stop=(ss == NS - 1),
                    )
            for t in range(3):
                pm = wp.tile([P, D], bf16, tag="pm")
                nc.scalar.activation(out=pm[:], in_=pooled[t][:, :],
                                     func=AF.Copy, scale=invc[:, t : t + 1])
                # z1 = relu(pm @ Wp1 + bp1)
                tp = ps.tile([D, P], bf16, space="PSUM", tag="aux")
                nc.tensor.transpose(out=tp[:, :], in_=pm[:], identity=i128b[:])
                pmT = wp.tile([D, P], bf16, tag="pmT")
                nc.vector.tensor_copy(out=pmT[:], in_=tp[:, :])
                z1p = psb.tile([P, D], f32, space="PSUM", tag="hn")
                nc.tensor.matmul(out=z1p[:, :], lhsT=pmT[:], rhs=Wt["Wp1"][:],
                                 start=True, stop=True)
                z1 = wp.tile([P, D], bf16, tag="z1")
                if has_bias:
                    ub2 = ps.tile([P, D], f32, space="PSUM", tag="aux")
                    nc.vector.tensor_tensor(
                        out=ub2[:, :], in0=z1p[:, :],
                        in1=bt["bp1"][:].to_broadcast([P, D]), op=ALU.add)
                    nc.scalar.activation(out=z1[:], in_=ub2[:, :], func=AF.Relu)
                else:
                    nc.scalar.activation(out=z1[:], in_=z1p[:, :], func=AF.Relu)
                tp2 = ps.tile([D, P], bf16, space="PSUM", tag="aux")
                nc.tensor.transpose(out=tp2[:, :], in_=z1[:], identity=i128b[:])
                z1T = wp.tile([D, P], bf16, tag="z1T")
                nc.vector.tensor_copy(out=z1T[:], in_=tp2[:, :])
                z2p = psb.tile([P, 32], f32, space="PSUM", tag="hn")
                nc.tensor.matmul(out=z2p[:, :], lhsT=z1T[:], rhs=Wt["Wp2"][:],
                                 start=True, stop=True)
                zo = wp.tile([P, 32], f32, tag="zo")
                if has_bias:
                    nc.vector.tensor_tensor(
                        out=zo[:], in0=z2p[:, :],
                        in1=bt["bp2"][:].to_broadcast([P, 32]), op=ALU.add)
                else:
                    nc.vector.tensor_copy(out=zo[:], in_=z2p[:, :])
                nc.sync.dma_start(out=z_out[t * P : (t + 1) * P, :], in_=zo[:])

    nc.compile()
    return nc

# ---------------------------------------------------------------------------
_CACHE = {}


def kernel(**inputs):
    x = np.asarray(inputs["x"], dtype=np.float32)
    edge_index = np.asarray(inputs["edge_index"]).astype(np.int64)
    batch = np.asarray(inputs["batch"]).astype(np.int64)
    G = 2500
    args = [np.asarray(inputs[k], dtype=np.float32) for k in
            ("W1", "W2", "W3", "Wp1", "Wp2", "b1", "b2", "b3", "bp1", "bp2")]
    W1, W2, W3, Wp1, Wp2, b1, b2, b3, bp1, bp2 = args
    has_bias = any(float(np.abs(b).max()) > 0 for b in (b1, b2, b3, bp1, bp2))

    key = hashlib.sha256(edge_index.tobytes() + batch.tobytes()).hexdigest()
    if key not in _CACHE:
        lay = build_layout(edge_index, batch, G=G)
        nc = build_program(lay, ncores=NCORES, has_bias=has_bias)
        _CACHE[key] = (lay, nc)
    lay, nc = _CACHE[key]

    ims = [core_inputs(lay, c, x, W1, W2, W3, Wp1, Wp2, b1, b2, b3, bp1, bp2)
           for c in range(NCORES)]
    res = run_bass_kernel_spmd(nc, ims, core_ids=list(range(NCORES)))

    z = np.zeros((G, 32), np.float32)
    for c in range(NCORES):
        gb, ge = lay["g_of_core"][c]
        z[gb:ge] = res.results[c]["z"][: ge - gb]
    return z



# revision 10
# speedup vs baseline: 1.2030x; 1.2030x over previous
"""Self-contained Trainium2 Bass kernel for nn_GNNEncoder (GCN message passing).

Strategy: partition graphs (and their node/edge slices) across 8 NeuronCores.
Each core owns a contiguous range of graphs; nodes are degree-sorted within
the core.  Per GCN layer: each core transforms its node slice (h = x @ W,
scaled by dis = 1/sqrt(deg)), AllGathers the bf16 node table, then runs the
full gather / segment-sum locally for its destination nodes using batched
dma_gather (256B rows) plus fixed selection-matrix matmuls on the PE array
(slot p -> psum row p//4).  Out-of-budget messages are aggregated through
"virtual node" partial sums (pass-2 staircase).  Mean-pooling per graph is a
matmul with a host-built one-hot membership matrix; the final MLP runs on
the pooled [ngraph, 64] tiles.  Output is assembled on the host.
"""
import sys

sys.path.insert(0, "/opt/trn_rl_repo")

import hashlib

import numpy as np

import concourse.bass as bass
import concourse.bacc as bacc
import concourse.tile as tile
from concourse import mybir
from concourse.bass_utils import run_bass_kernel_spmd
from concourse.tile_rust import add_dep_helper


NCORES = 8
FORCE_QUEUE = None  # set to an int to pin all SWDGE gathers to one queue (sim)
P = 128
D = 64
ELEM = 128          # bf16 elems per table row (256B)
WIN = 32768
GCAP = 384          # max graphs per core (3 tiles of 128)


def _wrap_idx(flat):
    """[num] -> [128, num/16] int16 wrapped: idx i at [i%16, i//16], tiled x8."""
    num = flat.size
    assert num % 16 == 0, num
    a = np.zeros((16, num // 16), dtype=np.int16)
    a[np.arange(num) % 16, np.arange(num) // 16] = flat.astype(np.int16)
    return np.tile(a, (8, 1))


def build_layout(edge_index, batch, G=2500):
    """Returns dict with global structure + per-core data arrays."""
    N = batch.shape[0]
    E = edge_index.shape[1]
    src_o, dst_o = np.asarray(edge_index[0]), np.asarray(edge_index[1])
    batch = np.asarray(batch)

    # ---- partition graphs across cores by balancing node counts ----
    gcnt = np.bincount(batch, minlength=G)          # nodes per graph
    gstart_node = np.concatenate([[0], np.cumsum(gcnt)])
    bounds = [0]
    for c in range(1, NCORES):
        target = round(N * c / NCORES)
        g = int(np.searchsorted(gstart_node, target))
        g = min(max(g, bounds[-1]), G)
        bounds.append(g)
    bounds.append(G)
    g_of_core = [(bounds[c], bounds[c + 1]) for c in range(NCORES)]
    n_c = [int(gstart_node[ge] - gstart_node[gb]) for gb, ge in g_of_core]
    ng_c = [ge - gb for gb, ge in g_of_core]
    assert max(ng_c) <= GCAP, ng_c

    nstrips = (max(n_c) + P - 1) // P
    CHUNK = (nstrips + 1) * P                        # +1 guaranteed zero strip
    R_TOT = NCORES * CHUNK
    win = min(WIN, R_TOT)
    HI_BASE = R_TOT - win

    # ---- in-degree (for sorting) ----
    indeg = np.bincount(dst_o, minlength=N)

    # ---- per-core node permutation: old node id -> (core, rank) ----
    node_core = np.empty(N, np.int32)
    for c, (gb, ge) in enumerate(g_of_core):
        node_core[gstart_node[gb]:gstart_node[ge]] = c
    new_gid = np.empty(N, np.int64)                  # old id -> new global id
    core_nodes_old = []                              # per core: old ids in rank order
    for c in range(NCORES):
        lo, hi = int(gstart_node[bounds[c]]), int(gstart_node[bounds[c + 1]])
        old_ids = np.arange(lo, hi)
        order = np.argsort(-indeg[old_ids], kind="stable")
        ranked = old_ids[order]
        core_nodes_old.append(ranked)
        new_gid[ranked] = c * CHUNK + np.arange(ranked.size)

    # ---- per-core edge lists bucketed by dst, split lo/hi by src new id ----
    src_n = new_gid[src_o]
    dst_c = node_core[dst_o]
    dst_rank = (new_gid[dst_o] % CHUNK).astype(np.int64)
    is_lo = src_n < win

    # per core: lists indexed by dst rank
    deg_lo = np.zeros((NCORES, nstrips * P), np.int32)
    deg_hi = np.zeros((NCORES, nstrips * P), np.int32)
    np.add.at(deg_lo, (dst_c, dst_rank), is_lo)
    np.add.at(deg_hi, (dst_c, dst_rank), ~is_lo)

    # sort edges by (core, dst_rank, lo/hi) for slot filling
    order = np.lexsort((~is_lo, dst_rank, dst_c))
    e_src = src_n[order]
    e_core = dst_c[order]
    e_rank = dst_rank[order]
    e_islo = is_lo[order]
    # per (core,dst): start offsets into sorted list
    tot_deg = deg_lo + deg_hi
    dst_off = np.zeros((NCORES, nstrips * P + 1), np.int64)
    for c in range(NCORES):
        dst_off[c, 1:] = np.cumsum(tot_deg[c])
        if c > 0:
            dst_off[c] += dst_off[c - 1, -1]

    # ---- choose per-strip budgets KL[s], KH[s] (uniform across cores) ----
    KL = np.zeros(nstrips, np.int32)
    KH = np.zeros(nstrips, np.int32)
    for s in range(nstrips):
        sl = slice(s * P, (s + 1) * P)
        for (deg, K) in ((deg_lo, KL), (deg_hi, KH)):
            d = deg[:, sl].ravel()                   # 8*128 counts
            best, bestc = 0, None
            for k in range(0, int(d.max()) + 4, 4):
                cost = NCORES * P * k + 3.0 * np.maximum(d - k, 0).sum()
                if bestc is None or cost < bestc:
                    best, bestc = k, cost
            K[s] = best
    NBL = KL // 4                                    # lo blocks per window
    NBH = KH // 4

    # ---- fill main slots + collect overflow (vnodes) ----
    lo_blocks_per_strip = NBL * 4                    # per strip (4 windows)
    hi_blocks_per_strip = NBH * 4
    CAP_LO = int(lo_blocks_per_strip.sum()) * P
    CAP_HI = int(hi_blocks_per_strip.sum()) * P
    CAP_VB = nstrips * 2 * P                         # 2 vnode blocks per strip

    lo_col0 = np.concatenate([[0], np.cumsum(lo_blocks_per_strip)])
    hi_col0 = np.concatenate([[0], np.cumsum(hi_blocks_per_strip)])
    CAP_LO = max(CAP_LO, P)
    CAP_HI = max(CAP_HI, P)

    idx_lo = np.zeros((NCORES, CAP_LO), np.int64)    # default 0 -> a pad row? see below
    idx_hi = np.zeros((NCORES, CAP_HI), np.int64)
    idx_vb = np.zeros((NCORES, CAP_VB), np.int64)    # 0 = T_vn zero row

    # zero rows: lo window: core0 chunk rows [n_c0, CHUNK) are zero; use CHUNK-1.
    ZLO = CHUNK - 1
    assert ZLO < win
    ZHI = R_TOT - 1 - HI_BASE                        # core7 last pad row, hi-window-relative
    idx_lo[:] = ZLO
    idx_hi[:] = ZHI

    # vnode assignment: per core, list of (strip, q, kind, msgs)
    vn_msgs = [[] for _ in range(NCORES)]            # per core: list of (count, [srcs], slotpos)
    for c in range(NCORES):
        for s in range(nstrips):
            kl, kh = int(KL[s]), int(KH[s])
            for r in range(P):
                dstr = s * P + r
                nlo, nhi = int(deg_lo[c, dstr]), int(deg_hi[c, dstr])
                if nlo + nhi == 0:
                    continue
                base = int(dst_off[c, dstr])
                srcs = e_src[base : base + nlo + nhi]
                w, q = divmod(r, 32)
                # lo slots
                take = min(nlo, kl)
                for k in range(take):
                    b, t = divmod(k, 4)
                    col = lo_col0[s] + w * (kl // 4) + b
                    idx_lo[c, col * P + q * 4 + t] = srcs[k]
                if nlo > kl:
                    vn_msgs[c].append((nlo - kl, srcs[kl:nlo], (s, r, 0)))
                # hi slots
                take = min(nhi, kh)
                for k in range(take):
                    b, t = divmod(k, 4)
                    col = hi_col0[s] + w * (kh // 4) + b
                    idx_hi[c, col * P + q * 4 + t] = srcs[nlo + k] - HI_BASE
                if nhi > kh:
                    vn_msgs[c].append((nhi - kh, srcs[nlo + kh:], (s, r, 1)))

    # ---- pass-2: vnodes sorted by count desc, staircase strips ----
    nvn = max((len(v) for v in vn_msgs), default=0)
    NVSTRIP = max(1, (nvn + P - 1) // P)
    # per vnode-strip: number of staircase blocks (uniform across cores)
    vb_counts = np.zeros((NCORES, NVSTRIP * P), np.int32)
    for c in range(NCORES):
        vn_msgs[c].sort(key=lambda x: -x[0])
        for i, (cnt, _, _) in enumerate(vn_msgs[c]):
            vb_counts[c, i] = cnt
    NSB = np.zeros(NVSTRIP, np.int32)                # staircase blocks per vstrip
    for v in range(NVSTRIP):
        NSB[v] = int(vb_counts[:, v * P : (v + 1) * P].max())
    # pass-2 gathers: lo-kind vnodes and hi-kind in the SAME staircase
    # (mixed sources!) -> need separate lo/hi passes. Instead: two separate
    # staircases would double machinery; simpler: one staircase but each
    # slot's source window differs per vnode kind -> impossible per call.
    # Resolution: sort vnodes by (kind, -count): lo-vnodes first. Then
    # per strip, per block: slots [0, n_lo_valid) from lo window and
    # [n_lo..] from hi window -> two calls with complementary zero-pads.
    for c in range(NCORES):
        vn_msgs[c].sort(key=lambda x: (x[2][2], -x[0]))
    vb_counts[:] = 0
    vkind = np.zeros((NCORES, NVSTRIP * P), np.int32)
    for c in range(NCORES):
        for i, (cnt, _, _) in enumerate(vn_msgs[c]):
            vb_counts[c, i] = cnt
            vkind[c, i] = vn_msgs[c][i][2][2]
    for v in range(NVSTRIP):
        NSB[v] = int(vb_counts[:, v * P : (v + 1) * P].max())
    CAP_P2 = int(NSB.sum()) * P
    idx_p2lo = np.full((NCORES, max(CAP_P2, 16)), ZLO, np.int64)
    idx_p2hi = np.full((NCORES, max(CAP_P2, 16)), ZHI, np.int64)
    p2_col0 = np.concatenate([[0], np.cumsum(NSB)])
    for c in range(NCORES):
        for i, (cnt, srcs, (s, r, kind)) in enumerate(vn_msgs[c]):
            v, p = divmod(i, P)
            for k in range(cnt):
                col = p2_col0[v] + k
                if kind == 0:
                    idx_p2lo[c, col * P + p] = srcs[k]
                else:
                    idx_p2hi[c, col * P + p] = srcs[k] - HI_BASE
            # main v-block slot for this vnode: T_vn row = 1 + i
            # strip s vnode blocks: cols [2s, 2s+2), slot p2 = 2*q + kind
            # where within-block: block = r//64, pos = (r%64)*2 + kind
            blk, rr = divmod(r, 64)
            idx_vb[c, (s * 2 + blk) * P + rr * 2 + kind] = 1 + i
    VCAP = NVSTRIP * P
    # pass-2 lo/hi column split: lo vnodes occupy leading rows (kind-major sort)
    last_lo_v, first_hi_v = -1, NVSTRIP
    for c in range(NCORES):
        for i, (cnt, _, (s_, r_, kind)) in enumerate(vn_msgs[c]):
            v = i // P
            if kind == 0:
                last_lo_v = max(last_lo_v, v)
            else:
                first_hi_v = min(first_hi_v, v)
    P2LO_NCOL = int(p2_col0[last_lo_v + 1]) if last_lo_v >= 0 else 0
    P2HI_COL0 = int(p2_col0[first_hi_v]) if first_hi_v < NVSTRIP else int(p2_col0[-1])

    # ---- degree / pooling data ----
    deg_arr = np.ones((NCORES, P, nstrips), np.float32)
    cnt_arr = np.ones((NCORES, P, 3), np.float32)
    spool = np.zeros((NCORES, P, nstrips * GCAP), np.float32)
    for c in range(NCORES):
        old = core_nodes_old[c]
        dg = (indeg[old] + 1).astype(np.float32)     # +1 self loop
        r = np.arange(old.size)
        deg_arr[c, r % P, r // P] = dg
        gb, ge = g_of_core[c]
        gl = (batch[old] - gb).astype(np.int64)      # local graph id per rank
        spool[c, r % P, (r // P) * GCAP + gl] = 1.0
        gcl = gcnt[gb:ge].astype(np.float32)
        gcl = np.maximum(gcl, 1.0)
        gi = np.arange(ge - gb)
        cnt_arr[c, gi % P, gi // P] = gcl

    return dict(
        N=N, G=G, NSTRIPS=nstrips, CHUNK=CHUNK, R_TOT=R_TOT, HI_BASE=HI_BASE, WIN=win,
        NBL=NBL, NBH=NBH, NVSTRIP=NVSTRIP, NSB=NSB, VCAP=VCAP,
        CAP_LO=CAP_LO, CAP_HI=CAP_HI, CAP_VB=CAP_VB, CAP_P2=max(CAP_P2, 16),
        P2LO_NCOL=P2LO_NCOL, P2HI_COL0=P2HI_COL0,
        lo_col0=lo_col0, hi_col0=hi_col0, p2_col0=p2_col0,
        g_of_core=g_of_core, ng_c=ng_c, n_c=n_c,
        core_nodes_old=core_nodes_old,
        idx_lo=idx_lo, idx_hi=idx_hi, idx_vb=idx_vb,
        idx_p2lo=idx_p2lo, idx_p2hi=idx_p2hi,
        deg=deg_arr, cnt=cnt_arr, spool=spool,
        wrap=_wrap_idx,
    )


def core_inputs(lay, c, x, W1, W2, W3, Wp1, Wp2, b1, b2, b3, bp1, bp2):
    """Build the in_map for core c (numpy arrays, host dtypes)."""
    import ml_dtypes
    bf = ml_dtypes.bfloat16
    CHUNK, nstrips = lay["CHUNK"], lay["NSTRIPS"]
    old = lay["core_nodes_old"][c]
    xs = np.zeros((CHUNK, 128), np.float32)
    xs[: old.size] = x[old]
    w = lay["wrap"]
    S4 = np.zeros((P, 32), bf)
    for p in range(P):
        S4[p, p // 4] = 1.0
    S2 = np.zeros((P, 64), bf)
    for p in range(P):
        S2[p, p // 2] = 1.0
    I128b = np.eye(P, dtype=bf)
    I128f = np.eye(P, dtype=np.float32)
    return {
        "x": xs,
        "deg": lay["deg"][c],
        "cnt": lay["cnt"][c],
        "spool": lay["spool"][c].astype(bf),
        "idx_lo": w(lay["idx_lo"][c]),
        "idx_hi": w(lay["idx_hi"][c]),
        "idx_vb": w(lay["idx_vb"][c]),
        "idx_p2lo": w(lay["idx_p2lo"][c]),
        "idx_p2hi": w(lay["idx_p2hi"][c]),
        "s4": S4, "s2": S2, "i128b": I128b, "i128f": I128f,
        "W1": W1.astype(bf), "W2": W2.astype(bf), "W3": W3.astype(bf),
        "Wp1": Wp1.astype(bf), "Wp2": Wp2.astype(bf),
        "b1": b1.reshape(1, -1).astype(np.float32),
        "b2": b2.reshape(1, -1).astype(np.float32),
        "b3": b3.reshape(1, -1).astype(np.float32),
        "bp1": bp1.reshape(1, -1).astype(np.float32),
        "bp2": bp2.reshape(1, -1).astype(np.float32),
    }




bf16 = mybir.dt.bfloat16
f32 = mybir.dt.float32
i16 = mybir.dt.int16

AF = mybir.ActivationFunctionType
ALU = mybir.AluOpType

MAIN_CHUNK = 4      # strips per main gather chunk
P2_CHUNK = 32       # max pass-2 columns per gather chunk


def build_program(lay, ncores=8, has_bias=False, reps=1, ablate=()):
    NS = lay["NSTRIPS"]
    CHUNK = lay["CHUNK"]
    R_TOT = lay["R_TOT"]
    HI_BASE = lay["HI_BASE"]
    NBL, NBH = lay["NBL"], lay["NBH"]
    NVS = lay["NVSTRIP"]
    NSB = lay["NSB"]
    CAP_LO, CAP_HI, CAP_VB, CAP_P2 = (
        lay["CAP_LO"], lay["CAP_HI"], lay["CAP_VB"], lay["CAP_P2"])
    WINE = lay["WIN"]
    LO_NCOL = lay["P2LO_NCOL"]
    HI_COL0 = lay["P2HI_COL0"]
    lo_col0, hi_col0, p2_col0 = lay["lo_col0"], lay["hi_col0"], lay["p2_col0"]
    NCOL_P2 = int(NSB.sum())
    VROWS = NVS * P
    p2_chunk = max(P2_CHUNK, int(NSB.max()) if NVS else 0)
    max_lo = max(int(lo_col0[min(s + MAIN_CHUNK, NS)] - lo_col0[s])
                 for s in range(0, NS, MAIN_CHUNK))
    max_hi = max(int(hi_col0[min(s + MAIN_CHUNK, NS)] - hi_col0[s])
                 for s in range(0, NS, MAIN_CHUNK))

    nc = bacc.Bacc("TRN2", target_bir_lowering=False, num_devices=ncores,
                   num_swdge_queues=4)

    # ---------------- I/O ----------------
    x_in = nc.dram_tensor("x", [CHUNK, 128], f32, kind="ExternalInput")
    deg_in = nc.dram_tensor("deg", [P, NS], f32, kind="ExternalInput")
    cnt_in = nc.dram_tensor("cnt", [P, 3], f32, kind="ExternalInput")
    spool_in = nc.dram_tensor("spool", [P, NS * GCAP], bf16, kind="ExternalInput")
    idx_lo_in = nc.dram_tensor("idx_lo", [P, CAP_LO // 16], i16, kind="ExternalInput")
    idx_hi_in = nc.dram_tensor("idx_hi", [P, CAP_HI // 16], i16, kind="ExternalInput")
    idx_vb_in = nc.dram_tensor("idx_vb", [P, CAP_VB // 16], i16, kind="ExternalInput")
    idx_p2lo_in = nc.dram_tensor("idx_p2lo", [P, CAP_P2 // 16], i16, kind="ExternalInput")
    idx_p2hi_in = nc.dram_tensor("idx_p2hi", [P, CAP_P2 // 16], i16, kind="ExternalInput")
    s4_in = nc.dram_tensor("s4", [P, 32], bf16, kind="ExternalInput")
    s2_in = nc.dram_tensor("s2", [P, 64], bf16, kind="ExternalInput")
    i128b_in = nc.dram_tensor("i128b", [P, P], bf16, kind="ExternalInput")
    i128f_in = nc.dram_tensor("i128f", [P, P], f32, kind="ExternalInput")
    W_in = {
        "W1": nc.dram_tensor("W1", [128, 64], bf16, kind="ExternalInput"),
        "W2": nc.dram_tensor("W2", [64, 64], bf16, kind="ExternalInput"),
        "W3": nc.dram_tensor("W3", [64, 64], bf16, kind="ExternalInput"),
        "Wp1": nc.dram_tensor("Wp1", [64, 64], bf16, kind="ExternalInput"),
        "Wp2": nc.dram_tensor("Wp2", [64, 32], bf16, kind="ExternalInput"),
    }
    b_in = {
        "b1": nc.dram_tensor("b1", [1, 64], f32, kind="ExternalInput"),
        "b2": nc.dram_tensor("b2", [1, 64], f32, kind="ExternalInput"),
        "b3": nc.dram_tensor("b3", [1, 64], f32, kind="ExternalInput"),
        "bp1": nc.dram_tensor("bp1", [1, 64], f32, kind="ExternalInput"),
        "bp2": nc.dram_tensor("bp2", [1, 32], f32, kind="ExternalInput"),
    }
    z_out = nc.dram_tensor("z", [GCAP, 32], f32, kind="ExternalOutput")

    # NOTE: only the first D (=64) elems of each 256B table row carry data —
    # every aggregation matmul reads rhs cols [0:D) — so the collective moves
    # compact 128B rows (T_slice/T_ag_c) and a local spread DMA writes them
    # into the 256B-pitch gather table (upper halves are never-read garbage).
    T_slice = nc.dram_tensor("T_slice", [CHUNK, ELEM], bf16)
    # double-buffered 256B-pitch shared table: the AllGather writes the
    # padded layout directly (upper 128B of each row is never-read garbage)
    T_ag2 = [nc.dram_tensor(f"T_ag{par}", [R_TOT, ELEM], bf16, addr_space="Shared")
             for par in range(2)]
    T_vn = nc.dram_tensor("T_vn", [1 + VROWS, ELEM], bf16)

    with tile.TileContext(nc) as tc:
        with (
            tc.tile_pool(name="const", bufs=1) as cp,
            tc.tile_pool(name="big", bufs=1) as bigp,
            tc.tile_pool(name="gat", bufs=3) as gp,
            tc.tile_pool(name="work", bufs=2) as wp,
            tc.tile_pool(name="ps", bufs=2, space="PSUM") as ps,
            tc.tile_pool(name="psb", bufs=1, space="PSUM") as psb,
            tc.tile_pool(name="pspool", bufs=1, space="PSUM") as psp,
        ):
            # ---------- load constants ----------
            def load(t_dram, shape, dtype, name):
                t = cp.tile(shape, dtype, tag=name)
                nc.sync.dma_start(out=t[:], in_=t_dram[:, :])
                return t

            idx_lo = load(idx_lo_in, [P, CAP_LO // 16], i16, "idxlo")
            idx_hi = load(idx_hi_in, [P, CAP_HI // 16], i16, "idxhi")
            idx_vb = load(idx_vb_in, [P, CAP_VB // 16], i16, "idxvb")
            idx_p2lo = load(idx_p2lo_in, [P, CAP_P2 // 16], i16, "idxp2lo")
            idx_p2hi = load(idx_p2hi_in, [P, CAP_P2 // 16], i16, "idxp2hi")
            s4 = load(s4_in, [P, 32], bf16, "s4")
            s2 = load(s2_in, [P, 64], bf16, "s2")
            i128b = load(i128b_in, [P, P], bf16, "i128b")
            i128f = load(i128f_in, [P, P], f32, "i128f")
            Wt = {k: load(v, [v.shape[0], v.shape[1]], bf16, k) for k, v in W_in.items()}
            bt = {k: load(v, [1, v.shape[1]], f32, k) for k, v in b_in.items()}
            deg = load(deg_in, [P, NS], f32, "deg")
            cnt = load(cnt_in, [P, 3], f32, "cnt")

            # dis = 1/sqrt(deg); invc = 1/cnt
            dtmp = cp.tile([P, NS], f32, tag="dtmp")
            nc.scalar.activation(out=dtmp[:], in_=deg[:], func=AF.Sqrt)
            dis = cp.tile([P, NS], f32, tag="dis")
            nc.vector.reciprocal(out=dis[:], in_=dtmp[:])
            invc = cp.tile([P, 3], f32, tag="invc")
            nc.vector.reciprocal(out=invc[:], in_=cnt[:])

            # big persistent buffers
            hsum = bigp.tile([P, NS * D], bf16, tag="hsum")
            nc.any.memset(hsum[:], 0.0)
            tstage = bigp.tile([P, NS * D], bf16, tag="tstage")
            nc.any.memset(tstage[:], 0.0)
            vzero = cp.tile([1, ELEM], bf16, tag="vzero")
            nc.any.memset(vzero[:], 0.0)
            nc.sync.dma_start(out=T_vn[0:1, :], in_=vzero[:])
            vtmp = bigp.tile([P, NVS * D], bf16, tag="vtmp")
            nc.any.memset(vtmp[:], 0.0)

            zpad = cp.tile([P, D], bf16, tag="zpad")
            nc.any.memset(zpad[:], 0.0)

            # collectives go through the dedicated CC pipeline (not SWDGE),
            # so all 4 SWDGE queues are available for immediate-mode gathers.
            # vb gathers wait on the whole pass-2 chain -> pin them to their
            # own queue so they never head-of-line-block the main gathers.
            qload = [0, 0, 0]
            VB_QUEUE = 3

            def next_q(n=1):
                if FORCE_QUEUE is not None:
                    return FORCE_QUEUE
                q = qload.index(min(qload))
                qload[q] += n
                return q

            def transform_strip(s, src_tile, src_slice, w_tile, fp32_in):
                """src rows [128 x k] -> tstage[:, s*ELEM : s*ELEM+64] = dis*(x@W)."""
                k = 128 if fp32_in else 64
                tp = ps.tile([k, 128], f32 if fp32_in else bf16, space="PSUM", tag="aux")
                nc.tensor.transpose(
                    out=tp[:, :], in_=src_tile[:, src_slice],
                    identity=(i128f if fp32_in else i128b)[:],
                )
                xT = wp.tile([k, 128], bf16, tag="xT")
                nc.vector.tensor_copy(out=xT[:], in_=tp[:, :])
                hn = psb.tile([P, D], f32, space="PSUM", tag="hn")
                nc.tensor.matmul(out=hn[:, :], lhsT=xT[:], rhs=w_tile[:],
                                 start=True, stop=True)
                nc.scalar.activation(
                    out=tstage[:, s * D : (s + 1) * D], in_=hn[:, :],
                    func=AF.Copy, scale=dis[:, s : s + 1],
                )

            for layer_it in range(3 * reps):
                layer = layer_it % 3 + 1
                T_ag = T_ag2[layer_it % 2]
                # ---------- phase A: build table (layer 1 only) ----------
                if layer_it == 0:
                    for s in range(NS):
                        xt = wp.tile([P, 128], f32, tag="xt")
                        nc.sync.dma_start(
                            out=xt[:], in_=x_in[s * P : (s + 1) * P, :])
                        transform_strip(s, xt, slice(0, 128), Wt["W1"], True)

                # ---------- phase B: export compact slice + AllGather ----------
                nc.sync.dma_start(
                    out=T_slice[0 : NS * P, 0:D].rearrange("(s p) c -> p s c", p=P),
                    in_=tstage[:].rearrange("p (s c) -> p s c", c=D),
                )
                if NS * P < CHUNK and layer_it == 0:
                    # zero the pad strip rows once (values persist)
                    nc.sync.dma_start(
                        out=T_slice[NS * P : CHUNK, 0:D]
                        .rearrange("(q p) c -> p q c", p=P),
                        in_=zpad[:].rearrange("p (q c) -> p q c", c=D)
                        .to_broadcast([P, (CHUNK - NS * P) // P, D]),
                    )
                if "nocc" not in ablate:
                    nc.gpsimd.collective_compute(
                        "AllGather", ALU.bypass,
                        ins=[T_slice[:, :]], outs=[T_ag[:, :]],
                        replica_groups=[list(range(ncores))],
                    )

                # ---------- phase C: pass-2 vnode partial sums ----------
                if NCOL_P2 > 0 and "novn" not in ablate:
                    # chunk pass-2 columns by vstrips
                    v = 0
                    while v < NVS:
                        v0 = v
                        cols0 = int(p2_col0[v0])
                        while v < NVS and (v == v0 or int(p2_col0[v + 1]) - cols0 <= p2_chunk):
                            v += 1
                        cols1 = int(p2_col0[v])
                        ncol = cols1 - cols0
                        if ncol == 0:
                            v += 1
                            continue
                        # lo part of these columns
                        lo_c0, lo_c1 = cols0, min(cols1, LO_NCOL)
                        hi_c0, hi_c1 = max(cols0, HI_COL0), cols1
                        buf_l = buf_h = None
                        if lo_c1 > lo_c0:
                            n = 1 if "tinygather" in ablate else (lo_c1 - lo_c0)
                            buf_l = gp.tile([P, p2_chunk * ELEM], bf16, tag="p2l")
                            nc.gpsimd.dma_gather(
                                out_ap=buf_l[:, : n * ELEM].rearrange(
                                    "p (n d) -> p n d", d=ELEM),
                                in_ap=T_ag[0:WINE, :],
                                idxs_ap=idx_p2lo[:, lo_c0 * 8 : (lo_c0 + n) * 8],
                                num_idxs=n * P, num_idxs_reg=n * P,
                                elem_size=ELEM, queue_num=next_q(n * P), single_packet=False,
                            )
                        if hi_c1 > hi_c0:
                            n = 1 if "tinygather" in ablate else (hi_c1 - hi_c0)
                            buf_h = gp.tile([P, p2_chunk * ELEM], bf16, tag="p2h")
                            nc.gpsimd.dma_gather(
                                out_ap=buf_h[:, : n * ELEM].rearrange(
                                    "p (n d) -> p n d", d=ELEM),
                                in_ap=T_ag[HI_BASE : HI_BASE + WINE, :],
                                idxs_ap=idx_p2hi[:, hi_c0 * 8 : (hi_c0 + n) * 8],
                                num_idxs=n * P, num_idxs_reg=n * P,
                                elem_size=ELEM, queue_num=next_q(n * P), single_packet=False,
                            )
                        for vv in range(v0, v):
                            nblk = int(NSB[vv])
                            if nblk == 0:
                                continue
                            vps = ps.tile([P, D], f32, space="PSUM", tag="acc")
                            first = True
                            mms = []
                            for k in range(nblk):
                                col = int(p2_col0[vv]) + k
                                if col < LO_NCOL:
                                    mms.append((buf_l, col - lo_c0))
                                if col >= HI_COL0:
                                    mms.append((buf_h, col - hi_c0))
                            for mi, (buf, rel) in enumerate(mms):
                                nc.tensor.matmul(
                                    out=vps[:, :], lhsT=i128b[:],
                                    rhs=buf[:, rel * ELEM : rel * ELEM + D],
                                    start=(mi == 0), stop=(mi == len(mms) - 1),
                                    skip_group_check=True,
                                )
                            nc.scalar.activation(
                                out=vtmp[:, vv * D : (vv + 1) * D], in_=vps[:, :],
                                func=AF.Copy,
                            )
                    nc.sync.dma_start(
                        out=T_vn[1 : 1 + VROWS, 0:D].rearrange(
                            "(v p) c -> p v c", p=P),
                        in_=vtmp[:].rearrange("p (v c) -> p v c", c=D),
                    )

                # ---------- phase D/E: main stream ----------
                s = 0
                while s < NS:
                    s0, s1 = s, min(s + MAIN_CHUNK, NS)
                    s = s1
                    lc0, lc1 = int(lo_col0[s0]), int(lo_col0[s1])
                    hc0, hc1 = int(hi_col0[s0]), int(hi_col0[s1])
                    vb0, vb1 = s0 * 2, s1 * 2
                    buf_lo = buf_hi = None
                    if lc1 > lc0:
                        n = 1 if "tinygather" in ablate else (lc1 - lc0)
                        buf_lo = gp.tile([P, max_lo * ELEM], bf16, tag="blo")
                        nc.gpsimd.dma_gather(
                            out_ap=buf_lo[:, : n * ELEM].rearrange(
                                "p (n d) -> p n d", d=ELEM),
                            in_ap=T_ag[0:WINE, :],
                            idxs_ap=idx_lo[:, lc0 * 8 : (lc0 + n) * 8],
                            num_idxs=n * P, num_idxs_reg=n * P,
                            elem_size=ELEM, queue_num=next_q(n * P), single_packet=False,
                        )
                    if hc1 > hc0:
                        n = 1 if "tinygather" in ablate else (hc1 - hc0)
                        buf_hi = gp.tile([P, max_hi * ELEM], bf16, tag="bhi")
                        nc.gpsimd.dma_gather(
                            out_ap=buf_hi[:, : n * ELEM].rearrange(
                                "p (n d) -> p n d", d=ELEM),
                            in_ap=T_ag[HI_BASE : HI_BASE + WINE, :],
                            idxs_ap=idx_hi[:, hc0 * 8 : (hc0 + n) * 8],
                            num_idxs=n * P, num_idxs_reg=n * P,
                            elem_size=ELEM, queue_num=next_q(n * P), single_packet=False,
                        )
                    buf_vb = gp.tile([P, MAIN_CHUNK * 2 * ELEM], bf16, tag="bvb")
                    if "novn" not in ablate:
                        n = 1 if "tinygather" in ablate else (vb1 - vb0)
                        nc.gpsimd.dma_gather(
                            out_ap=buf_vb[:, : n * ELEM].rearrange(
                                "p (n d) -> p n d", d=ELEM),
                            in_ap=T_vn[:, :],
                            idxs_ap=idx_vb[:, vb0 * 8 : (vb0 + n) * 8],
                            num_idxs=n * P, num_idxs_reg=n * P,
                            elem_size=ELEM, queue_num=VB_QUEUE, single_packet=False,
                        )

                    for ss in range(s0, s1):
                        nbl, nbh = int(NBL[ss]), int(NBH[ss])
                        acc = ps.tile([P, D], f32, space="PSUM", tag="acc")
                        for w in range(4):
                            total = nbl + nbh
                            done = 0
                            for b in range(nbl):
                                col = int(lo_col0[ss]) - lc0 + w * nbl + b
                                nc.tensor.matmul(
                                    out=acc[32 * w : 32 * w + 32, :], lhsT=s4[:],
                                    rhs=buf_lo[:, col * ELEM : col * ELEM + D],
                                    start=(done == 0), stop=(done == total - 1),
                                    tile_position=(0, 32 * w),
                                    skip_group_check=True,
                                )
                                done += 1
                            for b in range(nbh):
                                col = int(hi_col0[ss]) - hc0 + w * nbh + b
                                nc.tensor.matmul(
                                    out=acc[32 * w : 32 * w + 32, :], lhsT=s4[:],
                                    rhs=buf_hi[:, col * ELEM : col * ELEM + D],
                                    start=(done == 0), stop=(done == total - 1),
                                    tile_position=(0, 32 * w),
                                    skip_group_check=True,
                                )
                                done += 1
                        # vnode blocks (S2, 64-row windows)
                        for blk in range(2 if "novn" not in ablate else 0):
                            col = (ss - s0) * 2 + blk
                            nc.tensor.matmul(
                                out=acc[64 * blk : 64 * blk + 64, :], lhsT=s2[:],
                                rhs=buf_vb[:, col * ELEM : col * ELEM + D],
                                start=(nbl + nbh == 0), stop=True,
                                tile_position=(0, 64 * blk),
                                skip_group_check=True,
                            )
                        # self-loop add: acc += tstage strip
                        nc.vector.tensor_tensor(
                            out=acc[:, :], in0=acc[:, :],
                            in1=tstage[:, ss * D : (ss + 1) * D],
                            op=ALU.add,
                        )
                        # x_l = relu(dis * acc) [+ bias]
                        xl = wp.tile([P, D], bf16, tag="xl")
                        if has_bias:
                            ub = ps.tile([P, D], f32, space="PSUM", tag="aux")
                            nc.vector.scalar_tensor_tensor(
                                out=ub[:, :], in0=acc[:, :],
                                scalar=dis[:, ss : ss + 1],
                                in1=bt[f"b{layer}"][:].to_broadcast([P, D]),
                                op0=ALU.mult, op1=ALU.add,
                            )
                            nc.scalar.activation(out=xl[:], in_=ub[:, :], func=AF.Relu)
                        else:
                            nc.scalar.activation(
                                out=xl[:], in_=acc[:, :], func=AF.Relu,
                                scale=dis[:, ss : ss + 1],
                            )
                        # hsum += x_l
                        nc.vector.tensor_tensor(
                            out=hsum[:, ss * D : (ss + 1) * D],
                            in0=hsum[:, ss * D : (ss + 1) * D],
                            in1=xl[:], op=ALU.add,
                        )
                        # next-layer table entry
                        if layer < 3 or reps > 1:
                            transform_strip(
                                ss, xl, slice(0, D),
                                Wt["W2" if layer == 3 else f"W{layer + 1}"], False)

            # ---------- pooling ----------
            pooled = []
            for t in range(3):
                pt = psp.tile([P, D], f32, space="PSUM", tag=f"pool{t}")
                pooled.append(pt)
            for ss in range(NS):
                spt = wp.tile([P, GCAP], bf16, tag="spt")
                nc.sync.dma_start(out=spt[:], in_=spool_in[:, ss * GCAP : (ss + 1) * GCAP])
                for t in range(3):
                    nc.tensor.matmul(
                        out=pooled[t][:, :], lhsT=spt[:, t * P : (t + 1) * P],
                        rhs=hsum[:, ss * D : (ss + 1) * D],
                        start=(ss == 0), stop=(ss == NS - 1),
                    )
            for t in range(3):
                pm = wp.tile([P, D], bf16, tag="pm")
                nc.scalar.activation(out=pm[:], in_=pooled[t][:, :],
                                     func=AF.Copy, scale=invc[:, t : t + 1])
                # z1 = relu(pm @ Wp1 + bp1)
                tp = ps.tile([D, P], bf16, space="PSUM", tag="aux")
                nc.tensor.transpose(out=tp[:, :], in_=pm[:], identity=i128b[:])
                pmT = wp.tile([D, P], bf16, tag="pmT")
                nc.vector.tensor_copy(out=pmT[:], in_=tp[:, :])
                z1p = psb.tile([P, D], f32, space="PSUM", tag="hn")
                nc.tensor.matmul(out=z1p[:, :], lhsT=pmT[:], rhs=Wt["Wp1"][:],
                                 start=True, stop=True)
                z1 = wp.tile([P, D], bf16, tag="z1")
                if has_bias:
                    ub2 = ps.tile([P, D], f32, space="PSUM", tag="aux")
                    nc.vector.tensor_tensor(
                        out=ub2[:, :], in0=z1p[:, :],
                        in1=bt["bp1"][:].to_broadcast([P, D]), op=ALU.add)
                    nc.scalar.activation(out=z1[:], in_=ub2[:, :], func=AF.Relu)
                else:
                    nc.scalar.activation(out=z1[:], in_=z1p[:, :], func=AF.Relu)
                tp2 = ps.tile([D, P], bf16, space="PSUM", tag="aux")
                nc.tensor.transpose(out=tp2[:, :], in_=z1[:], identity=i128b[:])
                z1T = wp.tile([D, P], bf16, tag="z1T")
                nc.vector.tensor_copy(out=z1T[:], in_=tp2[:, :])
                z2p = psb.tile([P, 32], f32, space="PSUM", tag="hn")
                nc.tensor.matmul(out=z2p[:, :], lhsT=z1T[:], rhs=Wt["Wp2"][:],
                                 start=True, stop=True)
                zo = wp.tile([P, 32], f32, tag="zo")
                if has_bias:
                    nc.vector.tensor_tensor(
                        out=zo[:], in0=z2p[:, :],
                        in1=bt["bp2"][:].to_broadcast([P, 32]), op=ALU.add)
                else:
                    nc.vector.tensor_copy(out=zo[:], in_=z2p[:, :])
                nc.sync.dma_start(out=z_out[t * P : (t + 1) * P, :], in_=zo[:])

    nc.compile()
    return nc

# ---------------------------------------------------------------------------
_CACHE = {}


def kernel(**inputs):
    x = np.asarray(inputs["x"], dtype=np.float32)
    edge_index = np.asarray(inputs["edge_index"]).astype(np.int64)
    batch = np.asarray(inputs["batch"]).astype(np.int64)
    G = 2500
    args = [np.asarray(inputs[k], dtype=np.float32) for k in
            ("W1", "W2", "W3", "Wp1", "Wp2", "b1", "b2", "b3", "bp1", "bp2")]
    W1, W2, W3, Wp1, Wp2, b1, b2, b3, bp1, bp2 = args
    has_bias = any(float(np.abs(b).max()) > 0 for b in (b1, b2, b3, bp1, bp2))

    key = hashlib.sha256(edge_index.tobytes() + batch.tobytes()).hexdigest()
    if key not in _CACHE:
        lay = build_layout(edge_index, batch, G=G)
        nc = build_program(lay, ncores=NCORES, has_bias=has_bias)
        _CACHE[key] = (lay, nc)
    lay, nc = _CACHE[key]

    ims = [core_inputs(lay, c, x, W1, W2, W3, Wp1, Wp2, b1, b2, b3, bp1, bp2)
           for c in range(NCORES)]
    res = run_bass_kernel_spmd(nc, ims, core_ids=list(range(NCORES)))

    z = np.zeros((G, 32), np.float32)
    for c in range(NCORES):
        gb, ge = lay["g_of_core"][c]
        z[gb:ge] = res.results[c]["z"][: ge - gb]
    return z



# revision 12
# speedup vs baseline: 1.7953x; 1.4924x over previous
"""Self-contained Trainium2 Bass kernel for nn_GNNEncoder (GCN message passing).

Strategy: partition graphs (and their node/edge slices) across 8 NeuronCores.
Each core owns a contiguous range of graphs; nodes are degree-sorted within
the core.  Per GCN layer: each core transforms its node slice (h = x @ W,
scaled by dis = 1/sqrt(deg)), AllGathers the bf16 node table, then runs the
full gather / segment-sum locally for its destination nodes using batched
dma_gather (256B rows) plus fixed selection-matrix matmuls on the PE array
(slot p -> psum row p//4).  Out-of-budget messages are aggregated through
"virtual node" partial sums (pass-2 staircase).  Mean-pooling per graph is a
matmul with a host-built one-hot membership matrix; the final MLP runs on
the pooled [ngraph, 64] tiles.  Output is assembled on the host.
"""
import sys

sys.path.insert(0, "/opt/trn_rl_repo")

import hashlib

import numpy as np

import concourse.bass as bass
import concourse.bacc as bacc
import concourse.tile as tile
from concourse import mybir
from concourse.bass_utils import run_bass_kernel_spmd
from concourse.tile_rust import add_dep_helper


NCORES = 8
FORCE_QUEUE = None  # set to an int to pin all SWDGE gathers to one queue (sim)
P = 128
D = 64
ELEM = 128          # bf16 elems per table row (256B)
WIN = 32768
GCAP = 384          # max graphs per core (3 tiles of 128)


def _wrap_idx(flat):
    """[num] -> [128, num/16] int16 wrapped: idx i at [i%16, i//16], tiled x8."""
    num = flat.size
    assert num % 16 == 0, num
    a = np.zeros((16, num // 16), dtype=np.int16)
    a[np.arange(num) % 16, np.arange(num) // 16] = flat.astype(np.int16)
    return np.tile(a, (8, 1))


def build_layout(edge_index, batch, G=2500):
    """Returns dict with global structure + per-core data arrays."""
    N = batch.shape[0]
    E = edge_index.shape[1]
    src_o, dst_o = np.asarray(edge_index[0]), np.asarray(edge_index[1])
    batch = np.asarray(batch)

    # ---- partition graphs across cores by balancing node counts ----
    gcnt = np.bincount(batch, minlength=G)          # nodes per graph
    gstart_node = np.concatenate([[0], np.cumsum(gcnt)])
    bounds = [0]
    for c in range(1, NCORES):
        target = round(N * c / NCORES)
        g = int(np.searchsorted(gstart_node, target))
        g = min(max(g, bounds[-1]), G)
        bounds.append(g)
    bounds.append(G)
    g_of_core = [(bounds[c], bounds[c + 1]) for c in range(NCORES)]
    n_c = [int(gstart_node[ge] - gstart_node[gb]) for gb, ge in g_of_core]
    ng_c = [ge - gb for gb, ge in g_of_core]
    assert max(ng_c) <= GCAP, ng_c

    nstrips = (max(n_c) + P - 1) // P
    CHUNK = (nstrips + 1) * P                        # +1 guaranteed zero strip
    R_TOT = NCORES * CHUNK
    win = min(WIN, R_TOT)
    HI_BASE = R_TOT - win

    # ---- in-degree (for sorting) ----
    indeg = np.bincount(dst_o, minlength=N)

    # ---- per-core node permutation: old node id -> (core, rank) ----
    node_core = np.empty(N, np.int32)
    for c, (gb, ge) in enumerate(g_of_core):
        node_core[gstart_node[gb]:gstart_node[ge]] = c
    new_gid = np.empty(N, np.int64)                  # old id -> new global id
    core_nodes_old = []                              # per core: old ids in rank order
    for c in range(NCORES):
        lo, hi = int(gstart_node[bounds[c]]), int(gstart_node[bounds[c + 1]])
        old_ids = np.arange(lo, hi)
        order = np.argsort(-indeg[old_ids], kind="stable")
        ranked = old_ids[order]
        core_nodes_old.append(ranked)
        new_gid[ranked] = c * CHUNK + np.arange(ranked.size)

    # ---- per-core edge lists bucketed by dst, split lo/hi by src new id ----
    src_n = new_gid[src_o]
    dst_c = node_core[dst_o]
    dst_rank = (new_gid[dst_o] % CHUNK).astype(np.int64)
    is_lo = src_n < win

    # per core: lists indexed by dst rank
    deg_lo = np.zeros((NCORES, nstrips * P), np.int32)
    deg_hi = np.zeros((NCORES, nstrips * P), np.int32)
    np.add.at(deg_lo, (dst_c, dst_rank), is_lo)
    np.add.at(deg_hi, (dst_c, dst_rank), ~is_lo)

    # sort edges by (core, dst_rank, lo/hi) for slot filling
    order = np.lexsort((~is_lo, dst_rank, dst_c))
    e_src = src_n[order]
    e_core = dst_c[order]
    e_rank = dst_rank[order]
    e_islo = is_lo[order]
    # per (core,dst): start offsets into sorted list
    tot_deg = deg_lo + deg_hi
    dst_off = np.zeros((NCORES, nstrips * P + 1), np.int64)
    for c in range(NCORES):
        dst_off[c, 1:] = np.cumsum(tot_deg[c])
        if c > 0:
            dst_off[c] += dst_off[c - 1, -1]

    # ---- choose per-strip budgets KL[s], KH[s] (uniform across cores) ----
    KL = np.zeros(nstrips, np.int32)
    KH = np.zeros(nstrips, np.int32)
    for s in range(nstrips):
        sl = slice(s * P, (s + 1) * P)
        for (deg, K) in ((deg_lo, KL), (deg_hi, KH)):
            d = deg[:, sl].ravel()                   # 8*128 counts
            best, bestc = 0, None
            for k in range(0, int(d.max()) + 4, 4):
                cost = NCORES * P * k + 1.3 * np.maximum(d - k, 0).sum()
                if bestc is None or cost < bestc:
                    best, bestc = k, cost
            K[s] = best
    NBL = KL // 4                                    # lo blocks per window
    NBH = KH // 4

    # ---- fill main slots + collect overflow (vnodes) ----
    lo_blocks_per_strip = NBL * 4                    # per strip (4 windows)
    hi_blocks_per_strip = NBH * 4
    CAP_LO = int(lo_blocks_per_strip.sum()) * P
    CAP_HI = int(hi_blocks_per_strip.sum()) * P
    CAP_VB = nstrips * 2 * P                         # 2 vnode blocks per strip

    lo_col0 = np.concatenate([[0], np.cumsum(lo_blocks_per_strip)])
    hi_col0 = np.concatenate([[0], np.cumsum(hi_blocks_per_strip)])
    CAP_LO = max(CAP_LO, P)
    CAP_HI = max(CAP_HI, P)

    idx_lo = np.zeros((NCORES, CAP_LO), np.int64)    # default 0 -> a pad row? see below
    idx_hi = np.zeros((NCORES, CAP_HI), np.int64)
    idx_vb = np.zeros((NCORES, CAP_VB), np.int64)    # 0 = T_vn zero row

    # zero rows: lo window: core0 chunk rows [n_c0, CHUNK) are zero; use CHUNK-1.
    ZLO = CHUNK - 1
    assert ZLO < win
    ZHI = R_TOT - 1 - HI_BASE                        # core7 last pad row, hi-window-relative
    idx_lo[:] = ZLO
    idx_hi[:] = ZHI

    # vnode assignment: per core, list of (strip, q, kind, msgs)
    vn_msgs = [[] for _ in range(NCORES)]            # per core: list of (count, [srcs], slotpos)
    for c in range(NCORES):
        for s in range(nstrips):
            kl, kh = int(KL[s]), int(KH[s])
            for r in range(P):
                dstr = s * P + r
                nlo, nhi = int(deg_lo[c, dstr]), int(deg_hi[c, dstr])
                if nlo + nhi == 0:
                    continue
                base = int(dst_off[c, dstr])
                srcs = e_src[base : base + nlo + nhi]
                w, q = divmod(r, 32)
                # lo slots
                take = min(nlo, kl)
                for k in range(take):
                    b, t = divmod(k, 4)
                    col = lo_col0[s] + w * (kl // 4) + b
                    idx_lo[c, col * P + q * 4 + t] = srcs[k]
                if nlo > kl:
                    vn_msgs[c].append((nlo - kl, srcs[kl:nlo], (s, r, 0)))
                # hi slots
                take = min(nhi, kh)
                for k in range(take):
                    b, t = divmod(k, 4)
                    col = hi_col0[s] + w * (kh // 4) + b
                    idx_hi[c, col * P + q * 4 + t] = srcs[nlo + k] - HI_BASE
                if nhi > kh:
                    vn_msgs[c].append((nhi - kh, srcs[nlo + kh:], (s, r, 1)))

    # ---- pass-2: vnodes sorted by count desc, staircase strips ----
    nvn = max((len(v) for v in vn_msgs), default=0)
    NVSTRIP = max(1, (nvn + P - 1) // P)
    # per vnode-strip: number of staircase blocks (uniform across cores)
    vb_counts = np.zeros((NCORES, NVSTRIP * P), np.int32)
    for c in range(NCORES):
        vn_msgs[c].sort(key=lambda x: -x[0])
        for i, (cnt, _, _) in enumerate(vn_msgs[c]):
            vb_counts[c, i] = cnt
    NSB = np.zeros(NVSTRIP, np.int32)                # staircase blocks per vstrip
    for v in range(NVSTRIP):
        NSB[v] = int(vb_counts[:, v * P : (v + 1) * P].max())
    # pass-2 gathers: lo-kind vnodes and hi-kind in the SAME staircase
    # (mixed sources!) -> need separate lo/hi passes. Instead: two separate
    # staircases would double machinery; simpler: one staircase but each
    # slot's source window differs per vnode kind -> impossible per call.
    # Resolution: sort vnodes by (kind, -count): lo-vnodes first. Then
    # per strip, per block: slots [0, n_lo_valid) from lo window and
    # [n_lo..] from hi window -> two calls with complementary zero-pads.
    for c in range(NCORES):
        vn_msgs[c].sort(key=lambda x: (x[2][2], -x[0]))
    vb_counts[:] = 0
    vkind = np.zeros((NCORES, NVSTRIP * P), np.int32)
    for c in range(NCORES):
        for i, (cnt, _, _) in enumerate(vn_msgs[c]):
            vb_counts[c, i] = cnt
            vkind[c, i] = vn_msgs[c][i][2][2]
    for v in range(NVSTRIP):
        NSB[v] = int(vb_counts[:, v * P : (v + 1) * P].max())
    CAP_P2 = int(NSB.sum()) * P
    idx_p2lo = np.full((NCORES, max(CAP_P2, 16)), ZLO, np.int64)
    idx_p2hi = np.full((NCORES, max(CAP_P2, 16)), ZHI, np.int64)
    p2_col0 = np.concatenate([[0], np.cumsum(NSB)])
    for c in range(NCORES):
        for i, (cnt, srcs, (s, r, kind)) in enumerate(vn_msgs[c]):
            v, p = divmod(i, P)
            for k in range(cnt):
                col = p2_col0[v] + k
                if kind == 0:
                    idx_p2lo[c, col * P + p] = srcs[k]
                else:
                    idx_p2hi[c, col * P + p] = srcs[k] - HI_BASE
            # main v-block slot for this vnode: T_vn row = 1 + i
            # strip s vnode blocks: cols [2s, 2s+2), slot p2 = 2*q + kind
            # where within-block: block = r//64, pos = (r%64)*2 + kind
            blk, rr = divmod(r, 64)
            idx_vb[c, (s * 2 + blk) * P + rr * 2 + kind] = 1 + i
    VCAP = NVSTRIP * P
    # pass-2 lo/hi column split: lo vnodes occupy leading rows (kind-major sort)
    last_lo_v, first_hi_v = -1, NVSTRIP
    for c in range(NCORES):
        for i, (cnt, _, (s_, r_, kind)) in enumerate(vn_msgs[c]):
            v = i // P
            if kind == 0:
                last_lo_v = max(last_lo_v, v)
            else:
                first_hi_v = min(first_hi_v, v)
    P2LO_NCOL = int(p2_col0[last_lo_v + 1]) if last_lo_v >= 0 else 0
    P2HI_COL0 = int(p2_col0[first_hi_v]) if first_hi_v < NVSTRIP else int(p2_col0[-1])

    # ---- degree / pooling data ----
    deg_arr = np.ones((NCORES, P, nstrips), np.float32)
    cnt_arr = np.ones((NCORES, P, 3), np.float32)
    spool = np.zeros((NCORES, P, nstrips * GCAP), np.float32)
    for c in range(NCORES):
        old = core_nodes_old[c]
        dg = (indeg[old] + 1).astype(np.float32)     # +1 self loop
        r = np.arange(old.size)
        deg_arr[c, r % P, r // P] = dg
        gb, ge = g_of_core[c]
        gl = (batch[old] - gb).astype(np.int64)      # local graph id per rank
        spool[c, r % P, (r // P) * GCAP + gl] = 1.0
        gcl = gcnt[gb:ge].astype(np.float32)
        gcl = np.maximum(gcl, 1.0)
        gi = np.arange(ge - gb)
        cnt_arr[c, gi % P, gi // P] = gcl

    return dict(
        N=N, G=G, NSTRIPS=nstrips, CHUNK=CHUNK, R_TOT=R_TOT, HI_BASE=HI_BASE, WIN=win,
        NBL=NBL, NBH=NBH, NVSTRIP=NVSTRIP, NSB=NSB, VCAP=VCAP,
        CAP_LO=CAP_LO, CAP_HI=CAP_HI, CAP_VB=CAP_VB, CAP_P2=max(CAP_P2, 16),
        P2LO_NCOL=P2LO_NCOL, P2HI_COL0=P2HI_COL0,
        lo_col0=lo_col0, hi_col0=hi_col0, p2_col0=p2_col0,
        g_of_core=g_of_core, ng_c=ng_c, n_c=n_c,
        core_nodes_old=core_nodes_old,
        idx_lo=idx_lo, idx_hi=idx_hi, idx_vb=idx_vb,
        idx_p2lo=idx_p2lo, idx_p2hi=idx_p2hi,
        deg=deg_arr, cnt=cnt_arr, spool=spool,
        wrap=_wrap_idx,
    )


def core_inputs(lay, c, x, W1, W2, W3, Wp1, Wp2, b1, b2, b3, bp1, bp2):
    """Build the in_map for core c (numpy arrays, host dtypes)."""
    import ml_dtypes
    bf = ml_dtypes.bfloat16
    CHUNK, nstrips = lay["CHUNK"], lay["NSTRIPS"]
    old = lay["core_nodes_old"][c]
    xs = np.zeros((CHUNK, 128), np.float32)
    xs[: old.size] = x[old]
    w = lay["wrap"]
    S4 = np.zeros((P, 32), bf)
    for p in range(P):
        S4[p, p // 4] = 1.0
    S2 = np.zeros((P, 64), bf)
    for p in range(P):
        S2[p, p // 2] = 1.0
    I128b = np.eye(P, dtype=bf)
    I128f = np.eye(P, dtype=np.float32)
    return {
        "x": xs,
        "deg": lay["deg"][c],
        "cnt": lay["cnt"][c],
        "spool": lay["spool"][c].astype(bf),
        "idx_lo": w(lay["idx_lo"][c]),
        "idx_hi": w(lay["idx_hi"][c]),
        "idx_vb": w(lay["idx_vb"][c]),
        "idx_p2lo": w(lay["idx_p2lo"][c]),
        "idx_p2hi": w(lay["idx_p2hi"][c]),
        "s4": S4, "s2": S2, "i128b": I128b, "i128f": I128f,
        "W1": W1.astype(bf), "W2": W2.astype(bf), "W3": W3.astype(bf),
        "Wp1": Wp1.astype(bf), "Wp2": Wp2.astype(bf),
        "b1": b1.reshape(1, -1).astype(np.float32),
        "b2": b2.reshape(1, -1).astype(np.float32),
        "b3": b3.reshape(1, -1).astype(np.float32),
        "bp1": bp1.reshape(1, -1).astype(np.float32),
        "bp2": bp2.reshape(1, -1).astype(np.float32),
    }




bf16 = mybir.dt.bfloat16
f32 = mybir.dt.float32
i16 = mybir.dt.int16

AF = mybir.ActivationFunctionType
ALU = mybir.AluOpType

MAIN_CHUNK = 6      # strips per main gather chunk
P2_CHUNK = 32       # max pass-2 columns per gather chunk


def build_program(lay, ncores=8, has_bias=False, reps=1, ablate=()):
    NS = lay["NSTRIPS"]
    CHUNK = lay["CHUNK"]
    R_TOT = lay["R_TOT"]
    HI_BASE = lay["HI_BASE"]
    NBL, NBH = lay["NBL"], lay["NBH"]
    NVS = lay["NVSTRIP"]
    NSB = lay["NSB"]
    CAP_LO, CAP_HI, CAP_VB, CAP_P2 = (
        lay["CAP_LO"], lay["CAP_HI"], lay["CAP_VB"], lay["CAP_P2"])
    WINE = lay["WIN"]
    LO_NCOL = lay["P2LO_NCOL"]
    HI_COL0 = lay["P2HI_COL0"]
    lo_col0, hi_col0, p2_col0 = lay["lo_col0"], lay["hi_col0"], lay["p2_col0"]
    NCOL_P2 = int(NSB.sum())
    VROWS = NVS * P
    p2_chunk = max(P2_CHUNK, int(NSB.max()) if NVS else 0)
    max_lo = max(int(lo_col0[min(s + MAIN_CHUNK, NS)] - lo_col0[s])
                 for s in range(0, NS, MAIN_CHUNK))
    max_hi = max(int(hi_col0[min(s + MAIN_CHUNK, NS)] - hi_col0[s])
                 for s in range(0, NS, MAIN_CHUNK))

    nc = bacc.Bacc("TRN2", target_bir_lowering=False, num_devices=ncores,
                   num_swdge_queues=4)

    # ---------------- I/O ----------------
    x_in = nc.dram_tensor("x", [CHUNK, 128], f32, kind="ExternalInput")
    deg_in = nc.dram_tensor("deg", [P, NS], f32, kind="ExternalInput")
    cnt_in = nc.dram_tensor("cnt", [P, 3], f32, kind="ExternalInput")
    spool_in = nc.dram_tensor("spool", [P, NS * GCAP], bf16, kind="ExternalInput")
    idx_lo_in = nc.dram_tensor("idx_lo", [P, CAP_LO // 16], i16, kind="ExternalInput")
    idx_hi_in = nc.dram_tensor("idx_hi", [P, CAP_HI // 16], i16, kind="ExternalInput")
    idx_vb_in = nc.dram_tensor("idx_vb", [P, CAP_VB // 16], i16, kind="ExternalInput")
    idx_p2lo_in = nc.dram_tensor("idx_p2lo", [P, CAP_P2 // 16], i16, kind="ExternalInput")
    idx_p2hi_in = nc.dram_tensor("idx_p2hi", [P, CAP_P2 // 16], i16, kind="ExternalInput")
    s4_in = nc.dram_tensor("s4", [P, 32], bf16, kind="ExternalInput")
    s2_in = nc.dram_tensor("s2", [P, 64], bf16, kind="ExternalInput")
    i128b_in = nc.dram_tensor("i128b", [P, P], bf16, kind="ExternalInput")
    i128f_in = nc.dram_tensor("i128f", [P, P], f32, kind="ExternalInput")
    W_in = {
        "W1": nc.dram_tensor("W1", [128, 64], bf16, kind="ExternalInput"),
        "W2": nc.dram_tensor("W2", [64, 64], bf16, kind="ExternalInput"),
        "W3": nc.dram_tensor("W3", [64, 64], bf16, kind="ExternalInput"),
        "Wp1": nc.dram_tensor("Wp1", [64, 64], bf16, kind="ExternalInput"),
        "Wp2": nc.dram_tensor("Wp2", [64, 32], bf16, kind="ExternalInput"),
    }
    b_in = {
        "b1": nc.dram_tensor("b1", [1, 64], f32, kind="ExternalInput"),
        "b2": nc.dram_tensor("b2", [1, 64], f32, kind="ExternalInput"),
        "b3": nc.dram_tensor("b3", [1, 64], f32, kind="ExternalInput"),
        "bp1": nc.dram_tensor("bp1", [1, 64], f32, kind="ExternalInput"),
        "bp2": nc.dram_tensor("bp2", [1, 32], f32, kind="ExternalInput"),
    }
    z_out = nc.dram_tensor("z", [GCAP, 32], f32, kind="ExternalOutput")

    # NOTE: only the first D (=64) elems of each 256B table row carry data —
    # every aggregation matmul reads rhs cols [0:D) — so the collective moves
    # compact 128B rows (T_slice/T_ag_c) and a local spread DMA writes them
    # into the 256B-pitch gather table (upper halves are never-read garbage).
    T_slice = nc.dram_tensor("T_slice", [CHUNK, D], bf16)
    T_ag_c = nc.dram_tensor("T_ag_c", [R_TOT, D], bf16, addr_space="Shared")
    # double-buffered 256B-pitch table (gathers of layer L read parity L%2
    # while the next layer's spread writes the other parity)
    T_ag2 = [nc.dram_tensor(f"T_ag{par}", [R_TOT, ELEM], bf16) for par in range(2)]
    T_vn = nc.dram_tensor("T_vn", [1 + VROWS, ELEM], bf16)

    with tile.TileContext(nc) as tc:
        with (
            tc.tile_pool(name="const", bufs=1) as cp,
            tc.tile_pool(name="big", bufs=1) as bigp,
            tc.tile_pool(name="gat", bufs=3) as gp,
            tc.tile_pool(name="work", bufs=2) as wp,
            tc.tile_pool(name="ps", bufs=2, space="PSUM") as ps,
            tc.tile_pool(name="psb", bufs=1, space="PSUM") as psb,
            tc.tile_pool(name="pspool", bufs=1, space="PSUM") as psp,
        ):
            # ---------- load constants ----------
            def load(t_dram, shape, dtype, name):
                t = cp.tile(shape, dtype, tag=name)
                nc.sync.dma_start(out=t[:], in_=t_dram[:, :])
                return t

            idx_lo = load(idx_lo_in, [P, CAP_LO // 16], i16, "idxlo")
            idx_hi = load(idx_hi_in, [P, CAP_HI // 16], i16, "idxhi")
            idx_vb = load(idx_vb_in, [P, CAP_VB // 16], i16, "idxvb")
            idx_p2lo = load(idx_p2lo_in, [P, CAP_P2 // 16], i16, "idxp2lo")
            idx_p2hi = load(idx_p2hi_in, [P, CAP_P2 // 16], i16, "idxp2hi")
            s4 = load(s4_in, [P, 32], bf16, "s4")
            s2 = load(s2_in, [P, 64], bf16, "s2")
            i128b = load(i128b_in, [P, P], bf16, "i128b")
            i128f = load(i128f_in, [P, P], f32, "i128f")
            Wt = {k: load(v, [v.shape[0], v.shape[1]], bf16, k) for k, v in W_in.items()}
            bt = {k: load(v, [1, v.shape[1]], f32, k) for k, v in b_in.items()}
            deg = load(deg_in, [P, NS], f32, "deg")
            cnt = load(cnt_in, [P, 3], f32, "cnt")

            # dis = 1/sqrt(deg); invc = 1/cnt
            dtmp = cp.tile([P, NS], f32, tag="dtmp")
            nc.scalar.activation(out=dtmp[:], in_=deg[:], func=AF.Sqrt)
            dis = cp.tile([P, NS], f32, tag="dis")
            nc.vector.reciprocal(out=dis[:], in_=dtmp[:])
            invc = cp.tile([P, 3], f32, tag="invc")
            nc.vector.reciprocal(out=invc[:], in_=cnt[:])

            # big persistent buffers
            hsum = bigp.tile([P, NS * D], bf16, tag="hsum")
            nc.any.memset(hsum[:], 0.0)
            tstage = bigp.tile([P, NS * D], bf16, tag="tstage")
            nc.any.memset(tstage[:], 0.0)
            vzero = cp.tile([1, ELEM], bf16, tag="vzero")
            nc.any.memset(vzero[:], 0.0)
            nc.sync.dma_start(out=T_vn[0:1, :], in_=vzero[:])
            vtmp = bigp.tile([P, NVS * D], bf16, tag="vtmp")
            nc.any.memset(vtmp[:], 0.0)

            zpad = cp.tile([P, D], bf16, tag="zpad")
            nc.any.memset(zpad[:], 0.0)

            # collectives go through the dedicated CC pipeline (not SWDGE),
            # so all 4 SWDGE queues are available for immediate-mode gathers.
            # vb gathers wait on the whole pass-2 chain -> pin them to their
            # own queue so they never head-of-line-block the main gathers.
            qload = [0, 0, 0]
            VB_QUEUE = 3

            def next_q(n=1):
                if FORCE_QUEUE is not None:
                    return FORCE_QUEUE
                q = qload.index(min(qload))
                qload[q] += n
                return q

            def transform_strip(s, src_tile, src_slice, w_tile, fp32_in):
                """src rows [128 x k] -> tstage[:, s*ELEM : s*ELEM+64] = dis*(x@W)."""
                k = 128 if fp32_in else 64
                tp = ps.tile([k, 128], f32 if fp32_in else bf16, space="PSUM", tag="aux")
                nc.tensor.transpose(
                    out=tp[:, :], in_=src_tile[:, src_slice],
                    identity=(i128f if fp32_in else i128b)[:],
                )
                xT = wp.tile([k, 128], bf16, tag="xT")
                nc.vector.tensor_copy(out=xT[:], in_=tp[:, :])
                hn = psb.tile([P, D], f32, space="PSUM", tag="hn")
                nc.tensor.matmul(out=hn[:, :], lhsT=xT[:], rhs=w_tile[:],
                                 start=True, stop=True)
                nc.scalar.activation(
                    out=tstage[:, s * D : (s + 1) * D], in_=hn[:, :],
                    func=AF.Copy, scale=dis[:, s : s + 1],
                )

            for layer_it in range(3 * reps):
                layer = layer_it % 3 + 1
                T_ag = T_ag2[layer_it % 2]
                # ---------- phase A: build table (layer 1 only) ----------
                if layer_it == 0:
                    for s in range(NS):
                        xt = wp.tile([P, 128], f32, tag="xt")
                        nc.sync.dma_start(
                            out=xt[:], in_=x_in[s * P : (s + 1) * P, :])
                        transform_strip(s, xt, slice(0, 128), Wt["W1"], True)

                # ---------- phase B: export compact slice + AllGather ----------
                nc.sync.dma_start(
                    out=T_slice[0 : NS * P, :].rearrange("(s p) c -> p s c", p=P),
                    in_=tstage[:].rearrange("p (s c) -> p s c", c=D),
                )
                if NS * P < CHUNK and layer_it == 0:
                    # zero the pad strip rows once (values persist)
                    nc.sync.dma_start(
                        out=T_slice[NS * P : CHUNK, :]
                        .rearrange("(q p) c -> p q c", p=P),
                        in_=zpad[:].rearrange("p (q c) -> p q c", c=D)
                        .to_broadcast([P, (CHUNK - NS * P) // P, D]),
                    )
                if "nocc" not in ablate:
                    nc.gpsimd.collective_compute(
                        "AllGather", ALU.bypass,
                        ins=[T_slice[:, :]], outs=[T_ag_c[:, :]],
                        replica_groups=[list(range(ncores))],
                    )
                # spread compact rows into this layer's 256B-pitch table
                nc.sync.dma_start(out=T_ag[:, 0:D], in_=T_ag_c[:, :])

                # ---------- phase C: pass-2 vnode partial sums ----------
                if NCOL_P2 > 0 and "novn" not in ablate:
                    # chunk pass-2 columns by vstrips
                    v = 0
                    while v < NVS:
                        v0 = v
                        cols0 = int(p2_col0[v0])
                        while v < NVS and (v == v0 or int(p2_col0[v + 1]) - cols0 <= p2_chunk):
                            v += 1
                        cols1 = int(p2_col0[v])
                        ncol = cols1 - cols0
                        if ncol == 0:
                            v += 1
                            continue
                        # lo part of these columns
                        lo_c0, lo_c1 = cols0, min(cols1, LO_NCOL)
                        hi_c0, hi_c1 = max(cols0, HI_COL0), cols1
                        buf_l = buf_h = None
                        if lo_c1 > lo_c0:
                            n = 1 if "tinygather" in ablate else (lo_c1 - lo_c0)
                            buf_l = gp.tile([P, p2_chunk * ELEM], bf16, tag="p2l")
                            nc.gpsimd.dma_gather(
                                out_ap=buf_l[:, : n * ELEM].rearrange(
                                    "p (n d) -> p n d", d=ELEM),
                                in_ap=T_ag[0:WINE, :],
                                idxs_ap=idx_p2lo[:, lo_c0 * 8 : (lo_c0 + n) * 8],
                                num_idxs=n * P, num_idxs_reg=n * P,
                                elem_size=ELEM, queue_num=next_q(n * P), single_packet=False,
                            )
                        if hi_c1 > hi_c0:
                            n = 1 if "tinygather" in ablate else (hi_c1 - hi_c0)
                            buf_h = gp.tile([P, p2_chunk * ELEM], bf16, tag="p2h")
                            nc.gpsimd.dma_gather(
                                out_ap=buf_h[:, : n * ELEM].rearrange(
                                    "p (n d) -> p n d", d=ELEM),
                                in_ap=T_ag[HI_BASE : HI_BASE + WINE, :],
                                idxs_ap=idx_p2hi[:, hi_c0 * 8 : (hi_c0 + n) * 8],
                                num_idxs=n * P, num_idxs_reg=n * P,
                                elem_size=ELEM, queue_num=next_q(n * P), single_packet=False,
                            )
                        for vv in range(v0, v):
                            nblk = int(NSB[vv])
                            if nblk == 0:
                                continue
                            vps = ps.tile([P, D], f32, space="PSUM", tag="acc")
                            first = True
                            mms = []
                            for k in range(nblk):
                                col = int(p2_col0[vv]) + k
                                if col < LO_NCOL:
                                    mms.append((buf_l, col - lo_c0))
                                if col >= HI_COL0:
                                    mms.append((buf_h, col - hi_c0))
                            for mi, (buf, rel) in enumerate(mms):
                                nc.tensor.matmul(
                                    out=vps[:, :], lhsT=i128b[:],
                                    rhs=buf[:, rel * ELEM : rel * ELEM + D],
                                    start=(mi == 0), stop=(mi == len(mms) - 1),
                                    skip_group_check=True,
                                )
                            nc.scalar.activation(
                                out=vtmp[:, vv * D : (vv + 1) * D], in_=vps[:, :],
                                func=AF.Copy,
                            )
                    nc.sync.dma_start(
                        out=T_vn[1 : 1 + VROWS, 0:D].rearrange(
                            "(v p) c -> p v c", p=P),
                        in_=vtmp[:].rearrange("p (v c) -> p v c", c=D),
                    )

                # ---------- phase D/E: main stream ----------
                s = 0
                while s < NS:
                    s0, s1 = s, min(s + MAIN_CHUNK, NS)
                    s = s1
                    lc0, lc1 = int(lo_col0[s0]), int(lo_col0[s1])
                    hc0, hc1 = int(hi_col0[s0]), int(hi_col0[s1])
                    vb0, vb1 = s0 * 2, s1 * 2
                    buf_lo = buf_hi = None
                    if lc1 > lc0:
                        n = 1 if "tinygather" in ablate else (lc1 - lc0)
                        buf_lo = gp.tile([P, max_lo * ELEM], bf16, tag="blo")
                        nc.gpsimd.dma_gather(
                            out_ap=buf_lo[:, : n * ELEM].rearrange(
                                "p (n d) -> p n d", d=ELEM),
                            in_ap=T_ag[0:WINE, :],
                            idxs_ap=idx_lo[:, lc0 * 8 : (lc0 + n) * 8],
                            num_idxs=n * P, num_idxs_reg=n * P,
                            elem_size=ELEM, queue_num=next_q(n * P), single_packet=False,
                        )
                    if hc1 > hc0:
                        n = 1 if "tinygather" in ablate else (hc1 - hc0)
                        buf_hi = gp.tile([P, max_hi * ELEM], bf16, tag="bhi")
                        nc.gpsimd.dma_gather(
                            out_ap=buf_hi[:, : n * ELEM].rearrange(
                                "p (n d) -> p n d", d=ELEM),
                            in_ap=T_ag[HI_BASE : HI_BASE + WINE, :],
                            idxs_ap=idx_hi[:, hc0 * 8 : (hc0 + n) * 8],
                            num_idxs=n * P, num_idxs_reg=n * P,
                            elem_size=ELEM, queue_num=next_q(n * P), single_packet=False,
                        )
                    buf_vb = gp.tile([P, MAIN_CHUNK * 2 * ELEM], bf16, tag="bvb")
                    if "novn" not in ablate:
                        n = 1 if "tinygather" in ablate else (vb1 - vb0)
                        nc.gpsimd.dma_gather(
                            out_ap=buf_vb[:, : n * ELEM].rearrange(
                                "p (n d) -> p n d", d=ELEM),
                            in_ap=T_vn[:, :],
                            idxs_ap=idx_vb[:, vb0 * 8 : (vb0 + n) * 8],
                            num_idxs=n * P, num_idxs_reg=n * P,
                            elem_size=ELEM, queue_num=VB_QUEUE, single_packet=False,
                        )

                    for ss in range(s0, s1):
                        nbl, nbh = int(NBL[ss]), int(NBH[ss])
                        acc = ps.tile([P, D], f32, space="PSUM", tag="acc")
                        for w in range(4):
                            total = nbl + nbh
                            done = 0
                            for b in range(nbl):
                                col = int(lo_col0[ss]) - lc0 + w * nbl + b
                                nc.tensor.matmul(
                                    out=acc[32 * w : 32 * w + 32, :], lhsT=s4[:],
                                    rhs=buf_lo[:, col * ELEM : col * ELEM + D],
                                    start=(done == 0), stop=(done == total - 1),
                                    tile_position=(0, 32 * w),
                                    skip_group_check=True,
                                )
                                done += 1
                            for b in range(nbh):
                                col = int(hi_col0[ss]) - hc0 + w * nbh + b
                                nc.tensor.matmul(
                                    out=acc[32 * w : 32 * w + 32, :], lhsT=s4[:],
                                    rhs=buf_hi[:, col * ELEM : col * ELEM + D],
                                    start=(done == 0), stop=(done == total - 1),
                                    tile_position=(0, 32 * w),
                                    skip_group_check=True,
                                )
                                done += 1
                        # vnode blocks (S2, 64-row windows)
                        for blk in range(2 if "novn" not in ablate else 0):
                            col = (ss - s0) * 2 + blk
                            nc.tensor.matmul(
                                out=acc[64 * blk : 64 * blk + 64, :], lhsT=s2[:],
                                rhs=buf_vb[:, col * ELEM : col * ELEM + D],
                                start=(nbl + nbh == 0), stop=True,
                                tile_position=(0, 64 * blk),
                                skip_group_check=True,
                            )
                        # self-loop add: acc += tstage strip
                        nc.vector.tensor_tensor(
                            out=acc[:, :], in0=acc[:, :],
                            in1=tstage[:, ss * D : (ss + 1) * D],
                            op=ALU.add,
                        )
                        # x_l = relu(dis * acc) [+ bias]
                        xl = wp.tile([P, D], bf16, tag="xl")
                        if has_bias:
                            ub = ps.tile([P, D], f32, space="PSUM", tag="aux")
                            nc.vector.scalar_tensor_tensor(
                                out=ub[:, :], in0=acc[:, :],
                                scalar=dis[:, ss : ss + 1],
                                in1=bt[f"b{layer}"][:].to_broadcast([P, D]),
                                op0=ALU.mult, op1=ALU.add,
                            )
                            nc.scalar.activation(out=xl[:], in_=ub[:, :], func=AF.Relu)
                        else:
                            nc.scalar.activation(
                                out=xl[:], in_=acc[:, :], func=AF.Relu,
                                scale=dis[:, ss : ss + 1],
                            )
                        # hsum += x_l
                        nc.vector.tensor_tensor(
                            out=hsum[:, ss * D : (ss + 1) * D],
                            in0=hsum[:, ss * D : (ss + 1) * D],
                            in1=xl[:], op=ALU.add,
                        )
                        # next-layer table entry
                        if layer < 3 or reps > 1:
                            transform_strip(
                                ss, xl, slice(0, D),
                                Wt["W2" if layer == 3 else f"W{layer + 1}"], False)

            # ---------- pooling ----------
            pooled = []
            for t in range(3):
                pt = psp.tile([P, D], f32, space="PSUM", tag=f"pool{t}")
                pooled.append(pt)
            for ss in range(NS):
                spt = wp.tile([P, GCAP], bf16, tag="spt")
                nc.sync.dma_start(out=spt[:], in_=spool_in[:, ss * GCAP : (ss + 1) * GCAP])
                for t in range(3):
                    nc.tensor.matmul(
                        out=pooled[t][:, :], lhsT=spt[:, t * P : (t + 1) * P],
                        rhs=hsum[:, ss * D : (ss + 1) * D],
                        start=(ss == 0), stop=(ss == NS - 1),
                    )
            for t in range(3):
                pm = wp.tile([P, D], bf16, tag="pm")
                nc.scalar.activation(out=pm[:], in_=pooled[t][:, :],
                                     func=AF.Copy, scale=invc[:, t : t + 1])
                # z1 = relu(pm @ Wp1 + bp1)
                tp = ps.tile([D, P], bf16, space="PSUM", tag="aux")
                nc.tensor.transpose(out=tp[:, :], in_=pm[:], identity=i128b[:])
                pmT = wp.tile([D, P], bf16, tag="pmT")
                nc.vector.tensor_copy(out=pmT[:], in_=tp[:, :])
                z1p = psb.tile([P, D], f32, space="PSUM", tag="hn")
                nc.tensor.matmul(out=z1p[:, :], lhsT=pmT[:], rhs=Wt["Wp1"][:],
                                 start=True, stop=True)
                z1 = wp.tile([P, D], bf16, tag="z1")
                if has_bias:
                    ub2 = ps.tile([P, D], f32, space="PSUM", tag="aux")
                    nc.vector.tensor_tensor(
                        out=ub2[:, :], in0=z1p[:, :],
                        in1=bt["bp1"][:].to_broadcast([P, D]), op=ALU.add)
                    nc.scalar.activation(out=z1[:], in_=ub2[:, :], func=AF.Relu)
                else:
                    nc.scalar.activation(out=z1[:], in_=z1p[:, :], func=AF.Relu)
                tp2 = ps.tile([D, P], bf16, space="PSUM", tag="aux")
                nc.tensor.transpose(out=tp2[:, :], in_=z1[:], identity=i128b[:])
                z1T = wp.tile([D, P], bf16, tag="z1T")
                nc.vector.tensor_copy(out=z1T[:], in_=tp2[:, :])
                z2p = psb.tile([P, 32], f32, space="PSUM", tag="hn")
                nc.tensor.matmul(out=z2p[:, :], lhsT=z1T[:], rhs=Wt["Wp2"][:],
                                 start=True, stop=True)
                zo = wp.tile([P, 32], f32, tag="zo")
                if has_bias:
                    nc.vector.tensor_tensor(
                        out=zo[:], in0=z2p[:, :],
                        in1=bt["bp2"][:].to_broadcast([P, 32]), op=ALU.add)
                else:
                    nc.vector.tensor_copy(out=zo[:], in_=z2p[:, :])
                nc.sync.dma_start(out=z_out[t * P : (t + 1) * P, :], in_=zo[:])

    nc.compile()
    return nc

# ---------------------------------------------------------------------------
_CACHE = {}


def kernel(**inputs):
    x = np.asarray(inputs["x"], dtype=np.float32)
    edge_index = np.asarray(inputs["edge_index"]).astype(np.int64)
    batch = np.asarray(inputs["batch"]).astype(np.int64)
    G = 2500
    args = [np.asarray(inputs[k], dtype=np.float32) for k in
            ("W1", "W2", "W3", "Wp1", "Wp2", "b1", "b2", "b3", "bp1", "bp2")]
    W1, W2, W3, Wp1, Wp2, b1, b2, b3, bp1, bp2 = args
    has_bias = any(float(np.abs(b).max()) > 0 for b in (b1, b2, b3, bp1, bp2))

    key = hashlib.sha256(edge_index.tobytes() + batch.tobytes()).hexdigest()
    if key not in _CACHE:
        lay = build_layout(edge_index, batch, G=G)
        nc = build_program(lay, ncores=NCORES, has_bias=has_bias)
        _CACHE[key] = (lay, nc)
    lay, nc = _CACHE[key]

    ims = [core_inputs(lay, c, x, W1, W2, W3, Wp1, Wp2, b1, b2, b3, bp1, bp2)
           for c in range(NCORES)]
    res = run_bass_kernel_spmd(nc, ims, core_ids=list(range(NCORES)))

    z = np.zeros((G, 32), np.float32)
    for c in range(NCORES):
        gb, ge = lay["g_of_core"][c]
        z[gb:ge] = res.results[c]["z"][: ge - gb]
    return z



# revision 14
# speedup vs baseline: 1.8690x; 1.0410x over previous
"""Self-contained Trainium2 Bass kernel for nn_GNNEncoder (GCN message passing).

Strategy: partition graphs (and their node/edge slices) across 8 NeuronCores.
Each core owns a contiguous range of graphs; nodes are degree-sorted within
the core.  Per GCN layer: each core transforms its node slice (h = x @ W,
scaled by dis = 1/sqrt(deg)), AllGathers the bf16 node table, then runs the
full gather / segment-sum locally for its destination nodes using batched
dma_gather (256B rows) plus fixed selection-matrix matmuls on the PE array
(slot p -> psum row p//4).  Out-of-budget messages are aggregated through
"virtual node" partial sums (pass-2 staircase).  Mean-pooling per graph is a
matmul with a host-built one-hot membership matrix; the final MLP runs on
the pooled [ngraph, 64] tiles.  Output is assembled on the host.

Perf notes (measured on HW via reps-delta, 8 cores):
 - dma_gather is descriptor-rate bound (~9-10.7 ns/desc on one SWDGE queue;
   payload size is nearly free at 256B, and queue scaling is superlinear:
   3q ~2-3 ns/desc, 4q ~1.35 ns/desc in isolation).
 - main/pass-2 gathers round-robin queues 0-2; vnode-block gathers are
   pinned to queue 3 because they wait on the whole pass-2 chain (T_vn) and
   would otherwise head-of-line-block main gathers inside a shared ring.
 - overflow cost factor 1.3 (total descriptor count beats a smaller
   pass-2 system: lam=3 was ~25% slower end-to-end).
 - gather tile pool bufs=3 gives 12 strips of pipeline runway.
 - keeping the AllGather compact (128B rows) + a local 256B-pitch spread is
   ~2x faster than AllGathering the padded 256B-pitch table directly.
"""
import sys

sys.path.insert(0, "/opt/trn_rl_repo")

import hashlib

import numpy as np

import concourse.bass as bass
import concourse.bacc as bacc
import concourse.tile as tile
from concourse import mybir
from concourse.bass_utils import run_bass_kernel_spmd
from concourse.tile_rust import add_dep_helper


NCORES = 8
FORCE_QUEUE = None  # set to an int to pin all SWDGE gathers to one queue (sim)
P = 128
D = 64
ELEM = 128          # bf16 elems per table row (256B)
WIN = 32768
GCAP = 384          # max graphs per core (3 tiles of 128)


def _wrap_idx(flat):
    """[num] -> [128, num/16] int16 wrapped: idx i at [i%16, i//16], tiled x8."""
    num = flat.size
    assert num % 16 == 0, num
    a = np.zeros((16, num // 16), dtype=np.int16)
    a[np.arange(num) % 16, np.arange(num) // 16] = flat.astype(np.int16)
    return np.tile(a, (8, 1))


def build_layout(edge_index, batch, G=2500):
    """Returns dict with global structure + per-core data arrays."""
    N = batch.shape[0]
    E = edge_index.shape[1]
    src_o, dst_o = np.asarray(edge_index[0]), np.asarray(edge_index[1])
    batch = np.asarray(batch)

    # ---- partition graphs across cores by balancing node counts ----
    gcnt = np.bincount(batch, minlength=G)          # nodes per graph
    gstart_node = np.concatenate([[0], np.cumsum(gcnt)])
    bounds = [0]
    for c in range(1, NCORES):
        target = round(N * c / NCORES)
        g = int(np.searchsorted(gstart_node, target))
        g = min(max(g, bounds[-1]), G)
        bounds.append(g)
    bounds.append(G)
    g_of_core = [(bounds[c], bounds[c + 1]) for c in range(NCORES)]
    n_c = [int(gstart_node[ge] - gstart_node[gb]) for gb, ge in g_of_core]
    ng_c = [ge - gb for gb, ge in g_of_core]
    assert max(ng_c) <= GCAP, ng_c

    nstrips = (max(n_c) + P - 1) // P
    CHUNK = (nstrips + 1) * P                        # +1 guaranteed zero strip
    R_TOT = NCORES * CHUNK
    win = min(WIN, R_TOT)
    HI_BASE = R_TOT - win

    # ---- in-degree (for sorting) ----
    indeg = np.bincount(dst_o, minlength=N)

    # ---- per-core node permutation: old node id -> (core, rank) ----
    node_core = np.empty(N, np.int32)
    for c, (gb, ge) in enumerate(g_of_core):
        node_core[gstart_node[gb]:gstart_node[ge]] = c
    new_gid = np.empty(N, np.int64)                  # old id -> new global id
    core_nodes_old = []                              # per core: old ids in rank order
    for c in range(NCORES):
        lo, hi = int(gstart_node[bounds[c]]), int(gstart_node[bounds[c + 1]])
        old_ids = np.arange(lo, hi)
        order = np.argsort(-indeg[old_ids], kind="stable")
        ranked = old_ids[order]
        core_nodes_old.append(ranked)
        new_gid[ranked] = c * CHUNK + np.arange(ranked.size)

    # ---- per-core edge lists bucketed by dst, split lo/hi by src new id ----
    src_n = new_gid[src_o]
    dst_c = node_core[dst_o]
    dst_rank = (new_gid[dst_o] % CHUNK).astype(np.int64)
    is_lo = src_n < win

    # per core: lists indexed by dst rank
    deg_lo = np.zeros((NCORES, nstrips * P), np.int32)
    deg_hi = np.zeros((NCORES, nstrips * P), np.int32)
    np.add.at(deg_lo, (dst_c, dst_rank), is_lo)
    np.add.at(deg_hi, (dst_c, dst_rank), ~is_lo)

    # sort edges by (core, dst_rank, lo/hi) for slot filling
    order = np.lexsort((~is_lo, dst_rank, dst_c))
    e_src = src_n[order]
    e_core = dst_c[order]
    e_rank = dst_rank[order]
    e_islo = is_lo[order]
    # per (core,dst): start offsets into sorted list
    tot_deg = deg_lo + deg_hi
    dst_off = np.zeros((NCORES, nstrips * P + 1), np.int64)
    for c in range(NCORES):
        dst_off[c, 1:] = np.cumsum(tot_deg[c])
        if c > 0:
            dst_off[c] += dst_off[c - 1, -1]

    # ---- choose per-strip budgets KL[s], KH[s] (uniform across cores) ----
    KL = np.zeros(nstrips, np.int32)
    KH = np.zeros(nstrips, np.int32)
    for s in range(nstrips):
        sl = slice(s * P, (s + 1) * P)
        for (deg, K) in ((deg_lo, KL), (deg_hi, KH)):
            d = deg[:, sl].ravel()                   # 8*128 counts
            best, bestc = 0, None
            for k in range(0, int(d.max()) + 4, 4):
                cost = NCORES * P * k + 1.3 * np.maximum(d - k, 0).sum()
                if bestc is None or cost < bestc:
                    best, bestc = k, cost
            K[s] = best
    NBL = KL // 4                                    # lo blocks per window
    NBH = KH // 4

    # ---- fill main slots + collect overflow (vnodes) ----
    lo_blocks_per_strip = NBL * 4                    # per strip (4 windows)
    hi_blocks_per_strip = NBH * 4
    CAP_LO = int(lo_blocks_per_strip.sum()) * P
    CAP_HI = int(hi_blocks_per_strip.sum()) * P
    CAP_VB = nstrips * 2 * P                         # 2 vnode blocks per strip

    lo_col0 = np.concatenate([[0], np.cumsum(lo_blocks_per_strip)])
    hi_col0 = np.concatenate([[0], np.cumsum(hi_blocks_per_strip)])
    CAP_LO = max(CAP_LO, P)
    CAP_HI = max(CAP_HI, P)

    idx_lo = np.zeros((NCORES, CAP_LO), np.int64)    # default 0 -> a pad row? see below
    idx_hi = np.zeros((NCORES, CAP_HI), np.int64)
    idx_vb = np.zeros((NCORES, CAP_VB), np.int64)    # 0 = T_vn zero row

    # zero rows: lo window: core0 chunk rows [n_c0, CHUNK) are zero; use CHUNK-1.
    ZLO = CHUNK - 1
    assert ZLO < win
    ZHI = R_TOT - 1 - HI_BASE                        # core7 last pad row, hi-window-relative
    idx_lo[:] = ZLO
    idx_hi[:] = ZHI

    # vnode assignment: per core, list of (strip, q, kind, msgs)
    vn_msgs = [[] for _ in range(NCORES)]            # per core: list of (count, [srcs], slotpos)
    for c in range(NCORES):
        for s in range(nstrips):
            kl, kh = int(KL[s]), int(KH[s])
            for r in range(P):
                dstr = s * P + r
                nlo, nhi = int(deg_lo[c, dstr]), int(deg_hi[c, dstr])
                if nlo + nhi == 0:
                    continue
                base = int(dst_off[c, dstr])
                srcs = e_src[base : base + nlo + nhi]
                w, q = divmod(r, 32)
                # lo slots
                take = min(nlo, kl)
                for k in range(take):
                    b, t = divmod(k, 4)
                    col = lo_col0[s] + w * (kl // 4) + b
                    idx_lo[c, col * P + q * 4 + t] = srcs[k]
                if nlo > kl:
                    vn_msgs[c].append((nlo - kl, srcs[kl:nlo], (s, r, 0)))
                # hi slots
                take = min(nhi, kh)
                for k in range(take):
                    b, t = divmod(k, 4)
                    col = hi_col0[s] + w * (kh // 4) + b
                    idx_hi[c, col * P + q * 4 + t] = srcs[nlo + k] - HI_BASE
                if nhi > kh:
                    vn_msgs[c].append((nhi - kh, srcs[nlo + kh:], (s, r, 1)))

    # ---- pass-2: vnodes sorted by count desc, staircase strips ----
    nvn = max((len(v) for v in vn_msgs), default=0)
    NVSTRIP = max(1, (nvn + P - 1) // P)
    # per vnode-strip: number of staircase blocks (uniform across cores)
    vb_counts = np.zeros((NCORES, NVSTRIP * P), np.int32)
    for c in range(NCORES):
        vn_msgs[c].sort(key=lambda x: -x[0])
        for i, (cnt, _, _) in enumerate(vn_msgs[c]):
            vb_counts[c, i] = cnt
    NSB = np.zeros(NVSTRIP, np.int32)                # staircase blocks per vstrip
    for v in range(NVSTRIP):
        NSB[v] = int(vb_counts[:, v * P : (v + 1) * P].max())
    # pass-2 gathers: lo-kind vnodes and hi-kind in the SAME staircase
    # (mixed sources!) -> need separate lo/hi passes. Instead: two separate
    # staircases would double machinery; simpler: one staircase but each
    # slot's source window differs per vnode kind -> impossible per call.
    # Resolution: sort vnodes by (kind, -count): lo-vnodes first. Then
    # per strip, per block: slots [0, n_lo_valid) from lo window and
    # [n_lo..] from hi window -> two calls with complementary zero-pads.
    for c in range(NCORES):
        vn_msgs[c].sort(key=lambda x: (x[2][2], -x[0]))
    vb_counts[:] = 0
    vkind = np.zeros((NCORES, NVSTRIP * P), np.int32)
    for c in range(NCORES):
        for i, (cnt, _, _) in enumerate(vn_msgs[c]):
            vb_counts[c, i] = cnt
            vkind[c, i] = vn_msgs[c][i][2][2]
    for v in range(NVSTRIP):
        NSB[v] = int(vb_counts[:, v * P : (v + 1) * P].max())
    CAP_P2 = int(NSB.sum()) * P
    idx_p2lo = np.full((NCORES, max(CAP_P2, 16)), ZLO, np.int64)
    idx_p2hi = np.full((NCORES, max(CAP_P2, 16)), ZHI, np.int64)
    p2_col0 = np.concatenate([[0], np.cumsum(NSB)])
    for c in range(NCORES):
        for i, (cnt, srcs, (s, r, kind)) in enumerate(vn_msgs[c]):
            v, p = divmod(i, P)
            for k in range(cnt):
                col = p2_col0[v] + k
                if kind == 0:
                    idx_p2lo[c, col * P + p] = srcs[k]
                else:
                    idx_p2hi[c, col * P + p] = srcs[k] - HI_BASE
            # main v-block slot for this vnode: T_vn row = 1 + i
            # strip s vnode blocks: cols [2s, 2s+2), slot p2 = 2*q + kind
            # where within-block: block = r//64, pos = (r%64)*2 + kind
            blk, rr = divmod(r, 64)
            idx_vb[c, (s * 2 + blk) * P + rr * 2 + kind] = 1 + i
    VCAP = NVSTRIP * P
    # pass-2 lo/hi column split: lo vnodes occupy leading rows (kind-major sort)
    last_lo_v, first_hi_v = -1, NVSTRIP
    for c in range(NCORES):
        for i, (cnt, _, (s_, r_, kind)) in enumerate(vn_msgs[c]):
            v = i // P
            if kind == 0:
                last_lo_v = max(last_lo_v, v)
            else:
                first_hi_v = min(first_hi_v, v)
    P2LO_NCOL = int(p2_col0[last_lo_v + 1]) if last_lo_v >= 0 else 0
    P2HI_COL0 = int(p2_col0[first_hi_v]) if first_hi_v < NVSTRIP else int(p2_col0[-1])

    # ---- degree / pooling data ----
    deg_arr = np.ones((NCORES, P, nstrips), np.float32)
    cnt_arr = np.ones((NCORES, P, 3), np.float32)
    spool = np.zeros((NCORES, P, nstrips * GCAP), np.float32)
    for c in range(NCORES):
        old = core_nodes_old[c]
        dg = (indeg[old] + 1).astype(np.float32)     # +1 self loop
        r = np.arange(old.size)
        deg_arr[c, r % P, r // P] = dg
        gb, ge = g_of_core[c]
        gl = (batch[old] - gb).astype(np.int64)      # local graph id per rank
        spool[c, r % P, (r // P) * GCAP + gl] = 1.0
        gcl = gcnt[gb:ge].astype(np.float32)
        gcl = np.maximum(gcl, 1.0)
        gi = np.arange(ge - gb)
        cnt_arr[c, gi % P, gi // P] = gcl

    return dict(
        N=N, G=G, NSTRIPS=nstrips, CHUNK=CHUNK, R_TOT=R_TOT, HI_BASE=HI_BASE, WIN=win,
        NBL=NBL, NBH=NBH, NVSTRIP=NVSTRIP, NSB=NSB, VCAP=VCAP,
        CAP_LO=CAP_LO, CAP_HI=CAP_HI, CAP_VB=CAP_VB, CAP_P2=max(CAP_P2, 16),
        P2LO_NCOL=P2LO_NCOL, P2HI_COL0=P2HI_COL0,
        lo_col0=lo_col0, hi_col0=hi_col0, p2_col0=p2_col0,
        g_of_core=g_of_core, ng_c=ng_c, n_c=n_c,
        core_nodes_old=core_nodes_old,
        idx_lo=idx_lo, idx_hi=idx_hi, idx_vb=idx_vb,
        idx_p2lo=idx_p2lo, idx_p2hi=idx_p2hi,
        deg=deg_arr, cnt=cnt_arr, spool=spool,
        wrap=_wrap_idx,
    )


def core_inputs(lay, c, x, W1, W2, W3, Wp1, Wp2, b1, b2, b3, bp1, bp2):
    """Build the in_map for core c (numpy arrays, host dtypes)."""
    import ml_dtypes
    bf = ml_dtypes.bfloat16
    CHUNK, nstrips = lay["CHUNK"], lay["NSTRIPS"]
    old = lay["core_nodes_old"][c]
    xs = np.zeros((CHUNK, 128), np.float32)
    xs[: old.size] = x[old]
    w = lay["wrap"]
    S4 = np.zeros((P, 32), bf)
    for p in range(P):
        S4[p, p // 4] = 1.0
    S2 = np.zeros((P, 64), bf)
    for p in range(P):
        S2[p, p // 2] = 1.0
    I128b = np.eye(P, dtype=bf)
    I128f = np.eye(P, dtype=np.float32)
    return {
        "x": xs,
        "deg": lay["deg"][c],
        "cnt": lay["cnt"][c],
        "spool": lay["spool"][c].astype(bf),
        "idx_lo": w(lay["idx_lo"][c]),
        "idx_hi": w(lay["idx_hi"][c]),
        "idx_vb": w(lay["idx_vb"][c]),
        "idx_p2lo": w(lay["idx_p2lo"][c]),
        "idx_p2hi": w(lay["idx_p2hi"][c]),
        "s4": S4, "s2": S2, "i128b": I128b, "i128f": I128f,
        "W1": W1.astype(bf), "W2": W2.astype(bf), "W3": W3.astype(bf),
        "Wp1": Wp1.astype(bf), "Wp2": Wp2.astype(bf),
        "b1": b1.reshape(1, -1).astype(np.float32),
        "b2": b2.reshape(1, -1).astype(np.float32),
        "b3": b3.reshape(1, -1).astype(np.float32),
        "bp1": bp1.reshape(1, -1).astype(np.float32),
        "bp2": bp2.reshape(1, -1).astype(np.float32),
    }




bf16 = mybir.dt.bfloat16
f32 = mybir.dt.float32
i16 = mybir.dt.int16

AF = mybir.ActivationFunctionType
ALU = mybir.AluOpType

MAIN_CHUNK = 4      # strips per main gather chunk
P2_CHUNK = 32       # max pass-2 columns per gather chunk


def build_program(lay, ncores=8, has_bias=False, reps=1, ablate=()):
    NS = lay["NSTRIPS"]
    CHUNK = lay["CHUNK"]
    R_TOT = lay["R_TOT"]
    HI_BASE = lay["HI_BASE"]
    NBL, NBH = lay["NBL"], lay["NBH"]
    NVS = lay["NVSTRIP"]
    NSB = lay["NSB"]
    CAP_LO, CAP_HI, CAP_VB, CAP_P2 = (
        lay["CAP_LO"], lay["CAP_HI"], lay["CAP_VB"], lay["CAP_P2"])
    WINE = lay["WIN"]
    LO_NCOL = lay["P2LO_NCOL"]
    HI_COL0 = lay["P2HI_COL0"]
    lo_col0, hi_col0, p2_col0 = lay["lo_col0"], lay["hi_col0"], lay["p2_col0"]
    NCOL_P2 = int(NSB.sum())
    VROWS = NVS * P
    p2_chunk = max(P2_CHUNK, int(NSB.max()) if NVS else 0)
    max_lo = max(int(lo_col0[min(s + MAIN_CHUNK, NS)] - lo_col0[s])
                 for s in range(0, NS, MAIN_CHUNK))
    max_hi = max(int(hi_col0[min(s + MAIN_CHUNK, NS)] - hi_col0[s])
                 for s in range(0, NS, MAIN_CHUNK))

    nc = bacc.Bacc("TRN2", target_bir_lowering=False, num_devices=ncores,
                   num_swdge_queues=4)

    # ---------------- I/O ----------------
    x_in = nc.dram_tensor("x", [CHUNK, 128], f32, kind="ExternalInput")
    deg_in = nc.dram_tensor("deg", [P, NS], f32, kind="ExternalInput")
    cnt_in = nc.dram_tensor("cnt", [P, 3], f32, kind="ExternalInput")
    spool_in = nc.dram_tensor("spool", [P, NS * GCAP], bf16, kind="ExternalInput")
    idx_lo_in = nc.dram_tensor("idx_lo", [P, CAP_LO // 16], i16, kind="ExternalInput")
    idx_hi_in = nc.dram_tensor("idx_hi", [P, CAP_HI // 16], i16, kind="ExternalInput")
    idx_vb_in = nc.dram_tensor("idx_vb", [P, CAP_VB // 16], i16, kind="ExternalInput")
    idx_p2lo_in = nc.dram_tensor("idx_p2lo", [P, CAP_P2 // 16], i16, kind="ExternalInput")
    idx_p2hi_in = nc.dram_tensor("idx_p2hi", [P, CAP_P2 // 16], i16, kind="ExternalInput")
    s4_in = nc.dram_tensor("s4", [P, 32], bf16, kind="ExternalInput")
    s2_in = nc.dram_tensor("s2", [P, 64], bf16, kind="ExternalInput")
    i128b_in = nc.dram_tensor("i128b", [P, P], bf16, kind="ExternalInput")
    i128f_in = nc.dram_tensor("i128f", [P, P], f32, kind="ExternalInput")
    W_in = {
        "W1": nc.dram_tensor("W1", [128, 64], bf16, kind="ExternalInput"),
        "W2": nc.dram_tensor("W2", [64, 64], bf16, kind="ExternalInput"),
        "W3": nc.dram_tensor("W3", [64, 64], bf16, kind="ExternalInput"),
        "Wp1": nc.dram_tensor("Wp1", [64, 64], bf16, kind="ExternalInput"),
        "Wp2": nc.dram_tensor("Wp2", [64, 32], bf16, kind="ExternalInput"),
    }
    b_in = {
        "b1": nc.dram_tensor("b1", [1, 64], f32, kind="ExternalInput"),
        "b2": nc.dram_tensor("b2", [1, 64], f32, kind="ExternalInput"),
        "b3": nc.dram_tensor("b3", [1, 64], f32, kind="ExternalInput"),
        "bp1": nc.dram_tensor("bp1", [1, 64], f32, kind="ExternalInput"),
        "bp2": nc.dram_tensor("bp2", [1, 32], f32, kind="ExternalInput"),
    }
    z_out = nc.dram_tensor("z", [GCAP, 32], f32, kind="ExternalOutput")

    # NOTE: only the first D (=64) elems of each 256B table row carry data —
    # every aggregation matmul reads rhs cols [0:D) — so the collective moves
    # compact 128B rows (T_slice/T_ag_c) and a local spread DMA writes them
    # into the 256B-pitch gather table (upper halves are never-read garbage).
    T_slice = nc.dram_tensor("T_slice", [CHUNK, D], bf16)
    T_ag_c = nc.dram_tensor("T_ag_c", [R_TOT, D], bf16, addr_space="Shared")
    # double-buffered 256B-pitch table (gathers of layer L read parity L%2
    # while the next layer's spread writes the other parity)
    T_ag2 = [nc.dram_tensor(f"T_ag{par}", [R_TOT, ELEM], bf16) for par in range(2)]
    T_vn = nc.dram_tensor("T_vn", [1 + VROWS, ELEM], bf16)

    with tile.TileContext(nc) as tc:
        with (
            tc.tile_pool(name="const", bufs=1) as cp,
            tc.tile_pool(name="big", bufs=1) as bigp,
            tc.tile_pool(name="gat", bufs=3) as gp,
            tc.tile_pool(name="work", bufs=2) as wp,
            tc.tile_pool(name="ps", bufs=2, space="PSUM") as ps,
            tc.tile_pool(name="psb", bufs=1, space="PSUM") as psb,
            tc.tile_pool(name="pspool", bufs=1, space="PSUM") as psp,
        ):
            # ---------- load constants ----------
            def load(t_dram, shape, dtype, name):
                t = cp.tile(shape, dtype, tag=name)
                nc.sync.dma_start(out=t[:], in_=t_dram[:, :])
                return t

            idx_lo = load(idx_lo_in, [P, CAP_LO // 16], i16, "idxlo")
            idx_hi = load(idx_hi_in, [P, CAP_HI // 16], i16, "idxhi")
            idx_vb = load(idx_vb_in, [P, CAP_VB // 16], i16, "idxvb")
            idx_p2lo = load(idx_p2lo_in, [P, CAP_P2 // 16], i16, "idxp2lo")
            idx_p2hi = load(idx_p2hi_in, [P, CAP_P2 // 16], i16, "idxp2hi")
            s4 = load(s4_in, [P, 32], bf16, "s4")
            s2 = load(s2_in, [P, 64], bf16, "s2")
            i128b = load(i128b_in, [P, P], bf16, "i128b")
            i128f = load(i128f_in, [P, P], f32, "i128f")
            Wt = {k: load(v, [v.shape[0], v.shape[1]], bf16, k) for k, v in W_in.items()}
            bt = {k: load(v, [1, v.shape[1]], f32, k) for k, v in b_in.items()}
            deg = load(deg_in, [P, NS], f32, "deg")
            cnt = load(cnt_in, [P, 3], f32, "cnt")

            # dis = 1/sqrt(deg); invc = 1/cnt
            dtmp = cp.tile([P, NS], f32, tag="dtmp")
            nc.scalar.activation(out=dtmp[:], in_=deg[:], func=AF.Sqrt)
            dis = cp.tile([P, NS], f32, tag="dis")
            nc.vector.reciprocal(out=dis[:], in_=dtmp[:])
            invc = cp.tile([P, 3], f32, tag="invc")
            nc.vector.reciprocal(out=invc[:], in_=cnt[:])

            # big persistent buffers
            hsum = bigp.tile([P, NS * D], bf16, tag="hsum")
            nc.any.memset(hsum[:], 0.0)
            tstage = bigp.tile([P, NS * D], bf16, tag="tstage")
            nc.any.memset(tstage[:], 0.0)
            vzero = cp.tile([1, ELEM], bf16, tag="vzero")
            nc.any.memset(vzero[:], 0.0)
            nc.sync.dma_start(out=T_vn[0:1, :], in_=vzero[:])
            vtmp = bigp.tile([P, NVS * D], bf16, tag="vtmp")
            nc.any.memset(vtmp[:], 0.0)

            zpad = cp.tile([P, D], bf16, tag="zpad")
            nc.any.memset(zpad[:], 0.0)

            # collectives go through the dedicated CC pipeline (not SWDGE),
            # so all 4 SWDGE queues are available for immediate-mode gathers.
            # vb gathers wait on the whole pass-2 chain -> pin them to their
            # own queue so they never head-of-line-block the main gathers.
            qload = [0, 0, 0]
            VB_QUEUE = 3

            def next_q(n=1):
                if FORCE_QUEUE is not None:
                    return FORCE_QUEUE
                q = qload.index(min(qload))
                qload[q] += n
                return q

            def transform_strip(s, src_tile, src_slice, w_tile, fp32_in):
                """src rows [128 x k] -> tstage[:, s*ELEM : s*ELEM+64] = dis*(x@W)."""
                k = 128 if fp32_in else 64
                tp = ps.tile([k, 128], f32 if fp32_in else bf16, space="PSUM", tag="aux")
                nc.tensor.transpose(
                    out=tp[:, :], in_=src_tile[:, src_slice],
                    identity=(i128f if fp32_in else i128b)[:],
                )
                xT = wp.tile([k, 128], bf16, tag="xT")
                nc.vector.tensor_copy(out=xT[:], in_=tp[:, :])
                hn = psb.tile([P, D], f32, space="PSUM", tag="hn")
                nc.tensor.matmul(out=hn[:, :], lhsT=xT[:], rhs=w_tile[:],
                                 start=True, stop=True)
                nc.scalar.activation(
                    out=tstage[:, s * D : (s + 1) * D], in_=hn[:, :],
                    func=AF.Copy, scale=dis[:, s : s + 1],
                )

            for layer_it in range(3 * reps):
                layer = layer_it % 3 + 1
                T_ag = T_ag2[layer_it % 2]
                # ---------- phase A: build table (layer 1 only) ----------
                if layer_it == 0:
                    for s in range(NS):
                        xt = wp.tile([P, 128], f32, tag="xt")
                        nc.sync.dma_start(
                            out=xt[:], in_=x_in[s * P : (s + 1) * P, :])
                        transform_strip(s, xt, slice(0, 128), Wt["W1"], True)

                # ---------- phase B: export compact slice + AllGather ----------
                nc.sync.dma_start(
                    out=T_slice[0 : NS * P, :].rearrange("(s p) c -> p s c", p=P),
                    in_=tstage[:].rearrange("p (s c) -> p s c", c=D),
                )
                if NS * P < CHUNK and layer_it == 0:
                    # zero the pad strip rows once (values persist)
                    nc.sync.dma_start(
                        out=T_slice[NS * P : CHUNK, :]
                        .rearrange("(q p) c -> p q c", p=P),
                        in_=zpad[:].rearrange("p (q c) -> p q c", c=D)
                        .to_broadcast([P, (CHUNK - NS * P) // P, D]),
                    )
                if "nocc" not in ablate:
                    nc.gpsimd.collective_compute(
                        "AllGather", ALU.bypass,
                        ins=[T_slice[:, :]], outs=[T_ag_c[:, :]],
                        replica_groups=[list(range(ncores))],
                    )
                # spread compact rows into this layer's 256B-pitch table
                nc.sync.dma_start(out=T_ag[:, 0:D], in_=T_ag_c[:, :])

                # ---------- phase C: pass-2 vnode partial sums ----------
                if NCOL_P2 > 0 and "novn" not in ablate:
                    # chunk pass-2 columns by vstrips
                    v = 0
                    while v < NVS:
                        v0 = v
                        cols0 = int(p2_col0[v0])
                        while v < NVS and (v == v0 or int(p2_col0[v + 1]) - cols0 <= p2_chunk):
                            v += 1
                        cols1 = int(p2_col0[v])
                        ncol = cols1 - cols0
                        if ncol == 0:
                            v += 1
                            continue
                        # lo part of these columns
                        lo_c0, lo_c1 = cols0, min(cols1, LO_NCOL)
                        hi_c0, hi_c1 = max(cols0, HI_COL0), cols1
                        buf_l = buf_h = None
                        if lo_c1 > lo_c0:
                            n = 1 if "tinygather" in ablate else (lo_c1 - lo_c0)
                            buf_l = gp.tile([P, p2_chunk * ELEM], bf16, tag="p2l")
                            nc.gpsimd.dma_gather(
                                out_ap=buf_l[:, : n * ELEM].rearrange(
                                    "p (n d) -> p n d", d=ELEM),
                                in_ap=T_ag[0:WINE, :],
                                idxs_ap=idx_p2lo[:, lo_c0 * 8 : (lo_c0 + n) * 8],
                                num_idxs=n * P, num_idxs_reg=n * P,
                                elem_size=ELEM, queue_num=next_q(n * P), single_packet=False,
                            )
                        if hi_c1 > hi_c0:
                            n = 1 if "tinygather" in ablate else (hi_c1 - hi_c0)
                            buf_h = gp.tile([P, p2_chunk * ELEM], bf16, tag="p2h")
                            nc.gpsimd.dma_gather(
                                out_ap=buf_h[:, : n * ELEM].rearrange(
                                    "p (n d) -> p n d", d=ELEM),
                                in_ap=T_ag[HI_BASE : HI_BASE + WINE, :],
                                idxs_ap=idx_p2hi[:, hi_c0 * 8 : (hi_c0 + n) * 8],
                                num_idxs=n * P, num_idxs_reg=n * P,
                                elem_size=ELEM, queue_num=next_q(n * P), single_packet=False,
                            )
                        for vv in range(v0, v):
                            nblk = int(NSB[vv])
                            if nblk == 0:
                                continue
                            vps = ps.tile([P, D], f32, space="PSUM", tag="acc")
                            first = True
                            mms = []
                            for k in range(nblk):
                                col = int(p2_col0[vv]) + k
                                if col < LO_NCOL:
                                    mms.append((buf_l, col - lo_c0))
                                if col >= HI_COL0:
                                    mms.append((buf_h, col - hi_c0))
                            for mi, (buf, rel) in enumerate(mms):
                                nc.tensor.matmul(
                                    out=vps[:, :], lhsT=i128b[:],
                                    rhs=buf[:, rel * ELEM : rel * ELEM + D],
                                    start=(mi == 0), stop=(mi == len(mms) - 1),
                                    skip_group_check=True,
                                )
                            nc.scalar.activation(
                                out=vtmp[:, vv * D : (vv + 1) * D], in_=vps[:, :],
                                func=AF.Copy,
                            )
                    nc.sync.dma_start(
                        out=T_vn[1 : 1 + VROWS, 0:D].rearrange(
                            "(v p) c -> p v c", p=P),
                        in_=vtmp[:].rearrange("p (v c) -> p v c", c=D),
                    )

                # ---------- phase D/E: main stream ----------
                s = 0
                while s < NS:
                    s0, s1 = s, min(s + MAIN_CHUNK, NS)
                    s = s1
                    lc0, lc1 = int(lo_col0[s0]), int(lo_col0[s1])
                    hc0, hc1 = int(hi_col0[s0]), int(hi_col0[s1])
                    vb0, vb1 = s0 * 2, s1 * 2
                    buf_lo = buf_hi = None
                    if lc1 > lc0:
                        n = 1 if "tinygather" in ablate else (lc1 - lc0)
                        buf_lo = gp.tile([P, max_lo * ELEM], bf16, tag="blo")
                        nc.gpsimd.dma_gather(
                            out_ap=buf_lo[:, : n * ELEM].rearrange(
                                "p (n d) -> p n d", d=ELEM),
                            in_ap=T_ag[0:WINE, :],
                            idxs_ap=idx_lo[:, lc0 * 8 : (lc0 + n) * 8],
                            num_idxs=n * P, num_idxs_reg=n * P,
                            elem_size=ELEM, queue_num=next_q(n * P), single_packet=False,
                        )
                    if hc1 > hc0:
                        n = 1 if "tinygather" in ablate else (hc1 - hc0)
                        buf_hi = gp.tile([P, max_hi * ELEM], bf16, tag="bhi")
                        nc.gpsimd.dma_gather(
                            out_ap=buf_hi[:, : n * ELEM].rearrange(
                                "p (n d) -> p n d", d=ELEM),
                            in_ap=T_ag[HI_BASE : HI_BASE + WINE, :],
                            idxs_ap=idx_hi[:, hc0 * 8 : (hc0 + n) * 8],
                            num_idxs=n * P, num_idxs_reg=n * P,
                            elem_size=ELEM, queue_num=next_q(n * P), single_packet=False,
                        )
                    buf_vb = gp.tile([P, MAIN_CHUNK * 2 * ELEM], bf16, tag="bvb")
                    if "novn" not in ablate:
                        n = 1 if "tinygather" in ablate else (vb1 - vb0)
                        nc.gpsimd.dma_gather(
                            out_ap=buf_vb[:, : n * ELEM].rearrange(
                                "p (n d) -> p n d", d=ELEM),
                            in_ap=T_vn[:, :],
                            idxs_ap=idx_vb[:, vb0 * 8 : (vb0 + n) * 8],
                            num_idxs=n * P, num_idxs_reg=n * P,
                            elem_size=ELEM, queue_num=VB_QUEUE, single_packet=False,
                        )

                    for ss in range(s0, s1):
                        nbl, nbh = int(NBL[ss]), int(NBH[ss])
                        acc = ps.tile([P, D], f32, space="PSUM", tag="acc")
                        for w in range(4):
                            total = nbl + nbh
                            done = 0
                            for b in range(nbl):
                                col = int(lo_col0[ss]) - lc0 + w * nbl + b
                                nc.tensor.matmul(
                                    out=acc[32 * w : 32 * w + 32, :], lhsT=s4[:],
                                    rhs=buf_lo[:, col * ELEM : col * ELEM + D],
                                    start=(done == 0), stop=(done == total - 1),
                                    tile_position=(0, 32 * w),
                                    skip_group_check=True,
                                )
                                done += 1
                            for b in range(nbh):
                                col = int(hi_col0[ss]) - hc0 + w * nbh + b
                                nc.tensor.matmul(
                                    out=acc[32 * w : 32 * w + 32, :], lhsT=s4[:],
                                    rhs=buf_hi[:, col * ELEM : col * ELEM + D],
                                    start=(done == 0), stop=(done == total - 1),
                                    tile_position=(0, 32 * w),
                                    skip_group_check=True,
                                )
                                done += 1
                        # vnode blocks (S2, 64-row windows)
                        for blk in range(2 if "novn" not in ablate else 0):
                            col = (ss - s0) * 2 + blk
                            nc.tensor.matmul(
                                out=acc[64 * blk : 64 * blk + 64, :], lhsT=s2[:],
                                rhs=buf_vb[:, col * ELEM : col * ELEM + D],
                                start=(nbl + nbh == 0), stop=True,
                                tile_position=(0, 64 * blk),
                                skip_group_check=True,
                            )
                        # self-loop add: acc += tstage strip
                        nc.vector.tensor_tensor(
                            out=acc[:, :], in0=acc[:, :],
                            in1=tstage[:, ss * D : (ss + 1) * D],
                            op=ALU.add,
                        )
                        # x_l = relu(dis * acc) [+ bias]
                        xl = wp.tile([P, D], bf16, tag="xl")
                        if has_bias:
                            ub = ps.tile([P, D], f32, space="PSUM", tag="aux")
                            nc.vector.scalar_tensor_tensor(
                                out=ub[:, :], in0=acc[:, :],
                                scalar=dis[:, ss : ss + 1],
                                in1=bt[f"b{layer}"][:].to_broadcast([P, D]),
                                op0=ALU.mult, op1=ALU.add,
                            )
                            nc.scalar.activation(out=xl[:], in_=ub[:, :], func=AF.Relu)
                        else:
                            nc.scalar.activation(
                                out=xl[:], in_=acc[:, :], func=AF.Relu,
                                scale=dis[:, ss : ss + 1],
                            )
                        # hsum += x_l
                        nc.vector.tensor_tensor(
                            out=hsum[:, ss * D : (ss + 1) * D],
                            in0=hsum[:, ss * D : (ss + 1) * D],
                            in1=xl[:], op=ALU.add,
                        )
                        # next-layer table entry
                        if layer < 3 or reps > 1:
                            transform_strip(
                                ss, xl, slice(0, D),
                                Wt["W2" if layer == 3 else f"W{layer + 1}"], False)

            # ---------- pooling ----------
            pooled = []
            for t in range(3):
                pt = psp.tile([P, D], f32, space="PSUM", tag=f"pool{t}")
                pooled.append(pt)
            for ss in range(NS):
                spt = wp.tile([P, GCAP], bf16, tag="spt")
                nc.sync.dma_start(out=spt[:], in_=spool_in[:, ss * GCAP : (ss + 1) * GCAP])
                for t in range(3):
                    nc.tensor.matmul(
                        out=pooled[t][:, :], lhsT=spt[:, t * P : (t + 1) * P],
                        rhs=hsum[:, ss * D : (ss + 1) * D],
                        start=(ss == 0), stop=(ss == NS - 1),
                    )
            for t in range(3):
                pm = wp.tile([P, D], bf16, tag="pm")
                nc.scalar.activation(out=pm[:], in_=pooled[t][:, :],
                                     func=AF.Copy, scale=invc[:, t : t + 1])
                # z1 = relu(pm @ Wp1 + bp1)
                tp = ps.tile([D, P], bf16, space="PSUM", tag="aux")
                nc.tensor.transpose(out=tp[:, :], in_=pm[:], identity=i128b[:])
                pmT = wp.tile([D, P], bf16, tag="pmT")
                nc.vector.tensor_copy(out=pmT[:], in_=tp[:, :])
                z1p = psb.tile([P, D], f32, space="PSUM", tag="hn")
                nc.tensor.matmul(out=z1p[:, :], lhsT=pmT[:], rhs=Wt["Wp1"][:],
                                 start=True, stop=True)
                z1 = wp.tile([P, D], bf16, tag="z1")
                if has_bias:
                    ub2 = ps.tile([P, D], f32, space="PSUM", tag="aux")
                    nc.vector.tensor_tensor(
                        out=ub2[:, :], in0=z1p[:, :],
                        in1=bt["bp1"][:].to_broadcast([P, D]), op=ALU.add)
                    nc.scalar.activation(out=z1[:], in_=ub2[:, :], func=AF.Relu)
                else:
                    nc.scalar.activation(out=z1[:], in_=z1p[:, :], func=AF.Relu)
                tp2 = ps.tile([D, P], bf16, space="PSUM", tag="aux")
                nc.tensor.transpose(out=tp2[:, :], in_=z1[:], identity=i128b[:])
                z1T = wp.tile([D, P], bf16, tag="z1T")
                nc.vector.tensor_copy(out=z1T[:], in_=tp2[:, :])
                z2p = psb.tile([P, 32], f32, space="PSUM", tag="hn")
                nc.tensor.matmul(out=z2p[:, :], lhsT=z1T[:], rhs=Wt["Wp2"][:],
                                 start=True, stop=True)
                zo = wp.tile([P, 32], f32, tag="zo")
                if has_bias:
                    nc.vector.tensor_tensor(
                        out=zo[:], in0=z2p[:, :],
                        in1=bt["bp2"][:].to_broadcast([P, 32]), op=ALU.add)
                else:
                    nc.vector.tensor_copy(out=zo[:], in_=z2p[:, :])
                nc.sync.dma_start(out=z_out[t * P : (t + 1) * P, :], in_=zo[:])

    nc.compile()
    return nc

# ---------------------------------------------------------------------------
_CACHE = {}


def kernel(**inputs):
    x = np.asarray(inputs["x"], dtype=np.float32)
    edge_index = np.asarray(inputs["edge_index"]).astype(np.int64)
    batch = np.asarray(inputs["batch"]).astype(np.int64)
    G = 2500
    args = [np.asarray(inputs[k], dtype=np.float32) for k in
            ("W1", "W2", "W3", "Wp1", "Wp2", "b1", "b2", "b3", "bp1", "bp2")]
    W1, W2, W3, Wp1, Wp2, b1, b2, b3, bp1, bp2 = args
    has_bias = any(float(np.abs(b).max()) > 0 for b in (b1, b2, b3, bp1, bp2))

    key = hashlib.sha256(edge_index.tobytes() + batch.tobytes()).hexdigest()
    if key not in _CACHE:
        lay = build_layout(edge_index, batch, G=G)
        nc = build_program(lay, ncores=NCORES, has_bias=has_bias)
        _CACHE[key] = (lay, nc)
    lay, nc = _CACHE[key]

    ims = [core_inputs(lay, c, x, W1, W2, W3, Wp1, Wp2, b1, b2, b3, bp1, bp2)
           for c in range(NCORES)]
    res = run_bass_kernel_spmd(nc, ims, core_ids=list(range(NCORES)))

    z = np.zeros((G, 32), np.float32)
    for c in range(NCORES):
        gb, ge = lay["g_of_core"][c]
        z[gb:ge] = res.results[c]["z"][: ge - gb]
    return z

